# revision 1
# baseline (speedup 1.0000x reference)
"""MinibatchDiscrimination kernel for 8 Trainium2 NeuronCores.

reference:
    m = einsum('bi,iok->bok', x, T)          # B=128, IN=1024, OUT=512, K=16
    norm[i,j,o] = sum_k |m[j,o,k] - m[i,o,k]|
    o_b = sum_i exp(-norm) - 1               # [B, OUT]
    out = concat([x, o_b], axis=1)           # [128, 1536]

Sharding: each core owns OUT/8 = 64 output features (zero communication).

Per-core pipeline (pair-matmul, strictly-upper-triangular):
  1. GEMM on PE: m[b, f] = x @ T_c, f = o_local*16 + k (F = 1024, 8 f-tiles).
  2. Pair differences on PE: for f-tile t, diff[f, pair] = m_t.T @ psel where
     psel[b, (i,j)] = +1{b==i} - 1{b==j} over the 8128 pairs i<j. Streamed in
     [128, 512] PSUM chunks.
  3. |diff| -> SBUF bf16: ACT tiles use one Abs op per chunk; DVE tiles use
     two fused ops (relu(d), relu(-d)) into separate planes (the add is
     folded into the k-reduce contraction width).
  4. k-reduce + i-stacking on PE: per i one matmul over its pair block,
     selector S32_a [128, 32] with tile_position=(0, 32q) packs 16 i's into
     one [128, 128] group (row = 32*(isub//4) + 8*(isub%4) + osub); four
     groups share one PSUM bank [128, 512]; matmul start=True zeroes the
     bank once, so unwritten (j <= i) columns are exact zeros.
  5. exp(-norm) on ACT over [128, 512]; zeros exp to exactly 1.0 -> the
     deterministic junk is removed host-side (po[o,j] -= 128-j, rowsum -= i+1).
  6. Column sums: selector matmul S2_t [128, 64] accumulates over everything
     into PSUM [64, 128]. Row sums: DVE tensor_reduce -> [128, 64] table.
  7. Host: o_b[j, o] = (po[o, j] - (128-j)) + reindexed rowsums.
i==j pairs are never computed, so no "-1" correction is needed.
"""

import numpy as np
import ml_dtypes

import concourse.bass as bass
import concourse.tile as tile
from concourse import mybir
from concourse.bass_utils import run_bass_kernel_spmd

BF16 = mybir.dt.bfloat16
F32 = mybir.dt.float32
A = mybir.AluOpType
AF = mybir.ActivationFunctionType

B = 128
IN = 1024
OUT = 512
K = 16
NCORES = 8
OC = OUT // NCORES       # 64
F = OC * K               # 1024
NT = F // 128            # 8 f-tiles
NCI = IN // 128          # 8 contraction chunks
NPAIR = (B * (B - 1)) // 2   # 8128 strictly-upper pairs
CHUNK = 512
NCHUNK = (NPAIR + CHUNK - 1) // CHUNK   # 16 (last = 448)

# which f-tiles run their |diff| on DVE (two relu planes) vs ACT (one Abs op)
DVE_TILES = (False, False, False, False, False, False, False, False)
SUPER = 1024                                  # abs op width (2 PSUM banks)
NSUPER = (NPAIR + SUPER - 1) // SUPER         # 8 (last = 960)


def _pair_base(i):
    return i * 127 - (i * (i - 1)) // 2


def _split_excess_waits(nc, max_waits=1):
    """This walrus build rejects instructions carrying more than one sem
    wait; hoist extras onto preceding NoOps on the same engine."""
    for fn in nc.m.functions:
        for blk in fn.blocks:
            new_insts = []
            for inst in blk.instructions:
                si = inst.sync_info
                if si and si.on_wait and len(si.on_wait) > max_waits:
                    waits = list(si.on_wait)
                    extra, keep = waits[:-max_waits], waits[-max_waits:]
                    k = 0
                    while extra:
                        chunk, extra = extra[:max_waits], extra[max_waits:]
                        nop = mybir.InstNoOp(
                            name=f"{inst.name}-ws{k}", engine=inst.engine,
                            ins=[], outs=[],
                            sync_info=mybir.SyncInfo(on_wait=chunk, on_update=[]))
                        nc.register_instruction(nop)
                        new_insts.append(nop)
                        k += 1
                    inst.sync_info = mybir.SyncInfo(
                        on_wait=keep, on_update=list(si.on_update))
                new_insts.append(inst)
            blk.instructions[:] = new_insts


def _make_pd_abs_steps(nc, pools, t, m_bf, psel_sb):
    """Returns (absd_tile, steps): each step emits one pair-diff chunk
    matmul + its |.| op(s) when called."""
    work, ework, pdiff, pnorm = pools
    dve = DVE_TILES[t]
    planes = 2 if dve else 1
    absd = work.tile([128, planes, NPAIR], BF16, tag="absd")

    def step(c):
        lo = c * SUPER
        w = min(SUPER, NPAIR - lo)
        pd = pdiff.tile([128, SUPER], F32, tag="pd")
        # one matmul per PSUM bank (N <= 512), then one wide |.| op over
        # both banks to amortize the ACT/DVE per-op bubble
        for h in range(0, w, CHUNK):
            hw = min(CHUNK, w - h)
            nc.tensor.matmul(pd[:, h:h + hw], m_bf[:, 128 * t:128 * (t + 1)],
                             psel_sb[:, lo + h:lo + h + hw],
                             start=True, stop=True)
        if dve:
            nc.vector.tensor_scalar(absd[:, 0, lo:lo + w], pd[:, 0:w],
                                    0.0, None, op0=A.max)
            nc.vector.tensor_scalar(absd[:, 1, lo:lo + w], pd[:, 0:w],
                                    -1.0, 0.0, op0=A.mult, op1=A.max)
        else:
            nc.scalar.activation(absd[:, 0, lo:lo + w], pd[:, 0:w], AF.Abs)

    return absd, [lambda c=c: step(c) for c in range(NSUPER)]


def _emit_kred(nc, pools, t, absd, s32_sb, s2_sb, po, rs_all, weave=None):
    """k-reduce (packed, strip-interleaved) + exp + row/col sums for tile t.
    `weave` is a list of pending pair-diff steps for the NEXT tile; they are
    interleaved into the PE stream so the next tile's abs pass (ACT/DVE)
    overlaps this tile's k-reduce (PE)."""
    work, ework, pdiff, pnorm = pools
    dve = DVE_TILES[t]
    weave = list(weave or [])
    n_mm = 8 * 16 * (2 if dve else 1)
    stride = max(1, n_mm // (len(weave) + 1)) if weave else 0
    mm_count = 0

    def tick():
        nonlocal mm_count
        mm_count += 1
        if weave and stride and mm_count % stride == 0:
            weave.pop(0)()
    for G in range(2):
        pn = pnorm.tile([128, 512], F32, tag="pn")
        # zero the full tile: cells no matmul writes (j <= i) must read as
        # exact 0 so exp gives exactly 1.0 (host subtracts the known count)
        nc.vector.memset(pn[:], 0.0)
        first = True
        for gl in range(4):
            ig = 4 * G + gl
            for idx in range(16):
                # strip-interleaved: consecutive matmuls hit different
                # 32-col PE strips (q fastest) so they run concurrently
                q, a = idx % 4, idx // 4
                i = 16 * ig + 4 * a + q
                if i >= B - 1:
                    continue
                w = 127 - i
                bs = _pair_base(i)
                out_ap = pn[32 * q:32 * q + 32,
                            128 * gl + i + 1:128 * (gl + 1)]
                last = (gl == 3 and idx == 15)
                nc.tensor.matmul(
                    out_ap, s32_sb[a][:], absd[:, 0, bs:bs + w],
                    start=first, stop=(last and not dve),
                    tile_position=(0, 32 * q), skip_group_check=True)
                first = False
                tick()
                if dve:
                    # second relu plane accumulates into the same columns
                    nc.tensor.matmul(
                        out_ap, s32_sb[a][:], absd[:, 1, bs:bs + w],
                        start=False, stop=last,
                        tile_position=(0, 32 * q), skip_group_check=True)
                    tick()
        e = ework.tile([128, 512], BF16, tag="e")
        nc.scalar.activation(e[:], pn[:], AF.Exp, scale=-1.0)
        # row sums over j within each igroup -> rs_all[:, 8*ig + t]
        rs_view = rs_all.rearrange("p (ig tt) -> p ig tt", tt=8)
        nc.vector.tensor_reduce(
            rs_view[:, 4 * G:4 * G + 4, t],
            e[:].rearrange("p (g j) -> p g j", g=4), op=A.add,
            axis=mybir.AxisListType.X)
        for gl in range(4):
            ig = 4 * G + gl
            nc.tensor.matmul(po[:], s2_sb[t][:],
                             e[:, 128 * gl:128 * (gl + 1)],
                             start=(t == 0 and ig == 0),
                             stop=(t == NT - 1 and ig == 7))
    # flush any unwoven pair-diff steps for the next tile
    for stp in weave:
        stp()


def _build_program():
    nc = bass.Bass()
    xT_d = nc.dram_tensor("xt", [IN, B], BF16, kind="ExternalInput")
    tc_d = nc.dram_tensor("tc", [IN, F], BF16, kind="ExternalInput")
    psel_d = nc.dram_tensor("psel", [B, NPAIR], BF16, kind="ExternalInput")
    s32_d = nc.dram_tensor("s32", [4, 128, 32], BF16, kind="ExternalInput")
    s2_d = nc.dram_tensor("s2", [NT, 128, OC], BF16, kind="ExternalInput")
    po_d = nc.dram_tensor("po", [OC, B], F32, kind="ExternalOutput")
    rs_d = nc.dram_tensor("rs", [128, 64], F32, kind="ExternalOutput")

    with tile.TileContext(nc) as tc:
        with (
            tc.tile_pool(name="cst", bufs=1) as cst,
            tc.tile_pool(name="work", bufs=3) as work,
            tc.tile_pool(name="ework", bufs=4) as ework,
            tc.tile_pool(name="pgemm", bufs=1, space="PSUM") as pgemm,
            tc.tile_pool(name="pdiff", bufs=2, space="PSUM") as pdiff,
            tc.tile_pool(name="pnorm", bufs=2, space="PSUM") as pnorm,
            tc.tile_pool(name="pob", bufs=1, space="PSUM") as pob,
        ):
            xT_sb, tc_sb = [], []
            for ci in range(NCI):
                t_ = cst.tile([128, F], BF16, tag=f"tc{ci}")
                nc.sync.dma_start(t_[:], tc_d[128 * ci:128 * (ci + 1), :])
                tc_sb.append(t_)
                x_ = cst.tile([128, B], BF16, tag=f"xt{ci}")
                nc.sync.dma_start(x_[:], xT_d[128 * ci:128 * (ci + 1), :])
                xT_sb.append(x_)
            # per-chunk DMA so the first pair-diff matmul can start as soon
            # as its slice (and m_bf) lands, not after the full 2MB
            psel_sb = cst.tile([128, NPAIR], BF16, tag="psel")
            for cch in range(NCHUNK):
                lo = cch * CHUNK
                w = min(CHUNK, NPAIR - lo)
                nc.sync.dma_start(psel_sb[:, lo:lo + w], psel_d[:, lo:lo + w])
            s32_sb = []
            for a in range(4):
                t_ = cst.tile([128, 32], BF16, tag=f"s32_{a}")
                nc.sync.dma_start(t_[:], s32_d[a])
                s32_sb.append(t_)
            s2_sb = []
            for t in range(NT):
                t_ = cst.tile([128, OC], BF16, tag=f"s2{t}")
                nc.sync.dma_start(t_[:], s2_d[t])
                s2_sb.append(t_)

            # ---- GEMM: m[b, f] = x @ T_c ----
            m_bf = cst.tile([128, F], BF16, tag="mbf")
            for half in range(2):
                ps = pgemm.tile([128, 512], F32, tag="pg")
                for ci in range(NCI):
                    nc.tensor.matmul(
                        ps[:], xT_sb[ci][:],
                        tc_sb[ci][:, 512 * half:512 * (half + 1)],
                        start=(ci == 0), stop=(ci == NCI - 1))
                nc.scalar.activation(m_bf[:, 512 * half:512 * (half + 1)],
                                     ps[:], AF.Copy, scale=1.0)

            po = pob.tile([OC, B], F32, tag="po")
            rs_all = cst.tile([128, 64], F32, tag="rs")

            # software pipeline: tile t's k-reduce (PE) interleaves the
            # pair-diff chunks of tile t+1, so t+1's abs pass (ACT/DVE)
            # overlaps t's k-reduce instead of serializing after it
            pools = (work, ework, pdiff, pnorm)
            absd0, steps0 = _make_pd_abs_steps(nc, pools, 0, m_bf, psel_sb)
            for s in steps0:
                s()
            cur_absd = absd0
            for t in range(NT):
                if t + 1 < NT:
                    nxt_absd, nxt_steps = _make_pd_abs_steps(
                        nc, pools, t + 1, m_bf, psel_sb)
                else:
                    nxt_absd, nxt_steps = None, []
                _emit_kred(nc, pools, t, cur_absd, s32_sb, s2_sb,
                           po, rs_all, weave=nxt_steps)
                cur_absd = nxt_absd

            po_sb = cst.tile([OC, B], F32, tag="posb")
            nc.vector.tensor_copy(po_sb[:], po[:])
            nc.sync.dma_start(po_d[:], po_sb[:])
            nc.sync.dma_start(rs_d[:], rs_all[:])

    _split_excess_waits(nc)
    return nc


def _host_consts():
    psel = np.zeros((B, NPAIR), np.float32)
    col = 0
    for i in range(B - 1):
        w = 127 - i
        psel[i, col:col + w] = 1.0
        psel[np.arange(i + 1, 128), np.arange(col, col + w)] = -1.0
        col += w
    s32 = np.zeros((4, 128, 32), np.float32)
    for a in range(4):
        for osub in range(8):
            s32[a, 16 * osub:16 * (osub + 1), 8 * a + osub] = 1.0
    s2 = np.zeros((NT, 128, OC), np.float32)
    for t in range(NT):
        for p in range(128):
            s2[t, p, 8 * t + (p % 8)] = 1.0
    return (psel.astype(ml_dtypes.bfloat16), s32.astype(ml_dtypes.bfloat16),
            s2.astype(ml_dtypes.bfloat16))


_CACHE = {}


def _get_cached():
    if "nc" not in _CACHE:
        _CACHE["nc"] = _build_program()
        _CACHE["consts"] = _host_consts()
        # rowsum reindex: rs_all[p, 8*ig + t] belongs to
        # i = 16*ig + 4*a + q with p = 32*q + 8*a + osub, o = 8*t + osub
        p_idx = np.arange(128)
        q, rem = p_idx // 32, p_idx % 32
        a_, osub = rem // 8, rem % 8
        cols = np.arange(64)
        ig, t_ = cols // 8, cols % 8
        i_map = 16 * ig[None, :] + 4 * a_[:, None] + q[:, None]   # [128, 64]
        o_map = 8 * t_[None, :] + osub[:, None]                   # [128, 64]
        _CACHE["i_map"] = i_map
        _CACHE["o_map"] = o_map
    return _CACHE


def kernel(x: np.ndarray, T: np.ndarray, _trace=False, _tmpdir=None) -> np.ndarray:
    x = np.asarray(x, dtype=np.float32)
    T = np.asarray(T, dtype=np.float32)
    c = _get_cached()
    nc = c["nc"]
    psel, s32, s2 = c["consts"]

    xt = np.ascontiguousarray(x.T).astype(ml_dtypes.bfloat16)
    in_maps = []
    for cr in range(NCORES):
        tc_c = np.ascontiguousarray(
            T[:, OC * cr:OC * (cr + 1), :].reshape(IN, F)
        ).astype(ml_dtypes.bfloat16)
        in_maps.append({"xt": xt, "tc": tc_c, "psel": psel,
                        "s32": s32, "s2": s2})

    kw = {}
    if _trace:
        kw = dict(trace=True, tmpdir=_tmpdir)
    res = run_bass_kernel_spmd(nc, in_maps, list(range(NCORES)), **kw)

    jj = np.arange(B, dtype=np.float32)
    junk_col = (B - jj)[None, :]          # po[o, j] junk = 128 - j
    i_map, o_map = c["i_map"], c["o_map"]
    o_b = np.empty((B, OUT), np.float32)
    for cr in range(NCORES):
        r = res.results[cr]
        po = r["po"] - junk_col                       # [64, 128] colsums
        ob_c = po.T.copy()                            # [j, o_local]
        rows = r["rs"] - (i_map + 1)                  # rowsums minus junk
        np.add.at(ob_c, (i_map.ravel(), o_map.ravel()), rows.ravel())
        o_b[:, OC * cr:OC * (cr + 1)] = ob_c
    out = np.concatenate([x, o_b], axis=1)
    if _trace:
        return out, res
    return out



# revision 3
# speedup vs baseline: 1.0174x; 1.0174x over previous
"""MinibatchDiscrimination kernel for 8 Trainium2 NeuronCores.

reference:
    m = einsum('bi,iok->bok', x, T)          # B=128, IN=1024, OUT=512, K=16
    norm[i,j,o] = sum_k |m[j,o,k] - m[i,o,k]|
    o_b = sum_i exp(-norm) - 1               # [B, OUT]
    out = concat([x, o_b], axis=1)           # [128, 1536]

Sharding: each core owns OUT/8 = 64 output features (zero communication).

Per-core pipeline (pair-matmul, strictly-upper-triangular):
  1. GEMM on PE: m[b, f] = x @ (T_c/2), f = o_local*16 + k. Inputs are
     fp8e4 (T halved so values fit the fp8 range; the 2e-2 output
     tolerance plus exp(-norm) underflow makes the quantization free).
     PSUM -> SBUF fp8 copy on DVE.
  2. Pair differences on PE: diff[f, pair] = m_t.T @ psel (fp8 x fp8),
     psel[b, (i,j)] = +1{b==i} - 1{b==j} over the 8128 pairs i<j.
  3. |diff| -> SBUF bf16 on ACT, [128, 1024] supers.
  4. k-reduce + i-stacking on PE: per i one matmul, selector S32_a packs
     16 i's into a [128, 128] group via tile_position strips; four
     groups share one PSUM bank [128, 512]; unwritten (j <= i) cells
     stay exact zeros.
  5. exp(-2*norm_half) on ACT (the 2x undoes the T/2 scaling); zeros
     exp to exactly 1.0 -> deterministic junk removed host-side.
  6. Column sums: selector matmul S2_t accumulates into PSUM [64, 128].
     Row sums: DVE tensor_reduce -> [128, 64] table.
  7. Host: o_b[j, o] = (po[o, j] - (128-j)) + reindexed rowsums.
i==j pairs are never computed, so no "-1" correction is needed.

DMA notes: all inputs are fp8 (2.2 MB/core total) and loaded with
contiguous >=1KB-per-partition descriptors; tc (gates the GEMM) is
queued first, then x, then psel in 2KB column blocks.
"""

import numpy as np
import ml_dtypes

import concourse.bass as bass
import concourse.tile as tile
from concourse import mybir
from concourse.bass_utils import run_bass_kernel_spmd

BF16 = mybir.dt.bfloat16
F32 = mybir.dt.float32
FP8 = mybir.dt.float8e4
A = mybir.AluOpType
AF = mybir.ActivationFunctionType

B = 128
IN = 1024
OUT = 512
K = 16
NCORES = 8
OC = OUT // NCORES       # 64
F = OC * K               # 1024
NT = F // 128            # 8 f-tiles
NCI = IN // 128          # 8 contraction chunks
NPAIR = (B * (B - 1)) // 2   # 8128 strictly-upper pairs
CHUNK = 512
NCHUNK = (NPAIR + CHUNK - 1) // CHUNK   # 16 (last = 448)

# which f-tiles run their |diff| on DVE (two relu planes) vs ACT (one Abs op)
DVE_TILES = (False, False, False, False, False, False, False, False)
SUPER = 1024                                  # abs op width (2 PSUM banks)
NSUPER = (NPAIR + SUPER - 1) // SUPER         # 8 (last = 960)


def _pair_base(i):
    return i * 127 - (i * (i - 1)) // 2


def _split_excess_waits(nc, max_waits=1):
    """This walrus build rejects instructions carrying more than one sem
    wait; hoist extras onto preceding NoOps on the same engine."""
    for fn in nc.m.functions:
        for blk in fn.blocks:
            new_insts = []
            for inst in blk.instructions:
                si = inst.sync_info
                if si and si.on_wait and len(si.on_wait) > max_waits:
                    waits = list(si.on_wait)
                    extra, keep = waits[:-max_waits], waits[-max_waits:]
                    k = 0
                    while extra:
                        chunk, extra = extra[:max_waits], extra[max_waits:]
                        nop = mybir.InstNoOp(
                            name=f"{inst.name}-ws{k}", engine=inst.engine,
                            ins=[], outs=[],
                            sync_info=mybir.SyncInfo(on_wait=chunk, on_update=[]))
                        nc.register_instruction(nop)
                        new_insts.append(nop)
                        k += 1
                    inst.sync_info = mybir.SyncInfo(
                        on_wait=keep, on_update=list(si.on_update))
                new_insts.append(inst)
            blk.instructions[:] = new_insts


def _make_pd_abs_steps(nc, pools, t, m_bf, psel_sb):
    """Returns (absd_tile, steps): each step emits one pair-diff chunk
    matmul + its |.| op(s) when called."""
    work, ework, pdiff, pnorm = pools
    dve = DVE_TILES[t]
    planes = 2 if dve else 1
    absd = work.tile([128, planes, NPAIR], BF16, tag="absd")

    def step(c):
        lo = c * SUPER
        w = min(SUPER, NPAIR - lo)
        pd = pdiff.tile([128, SUPER], F32, tag="pd")
        # one matmul per PSUM bank (N <= 512), then one wide |.| op over
        # both banks to amortize the ACT/DVE per-op bubble
        for h in range(0, w, CHUNK):
            hw = min(CHUNK, w - h)
            nc.tensor.matmul(pd[:, h:h + hw], m_bf[:, 128 * t:128 * (t + 1)],
                             psel_sb[:, lo + h:lo + h + hw],
                             start=True, stop=True)
        if dve:
            nc.vector.tensor_scalar(absd[:, 0, lo:lo + w], pd[:, 0:w],
                                    0.0, None, op0=A.max)
            nc.vector.tensor_scalar(absd[:, 1, lo:lo + w], pd[:, 0:w],
                                    -1.0, 0.0, op0=A.mult, op1=A.max)
        else:
            nc.scalar.activation(absd[:, 0, lo:lo + w], pd[:, 0:w], AF.Abs)

    return absd, [lambda c=c: step(c) for c in range(NSUPER)]


def _emit_kred(nc, pools, t, absd, s32_sb, s2_sb, po, rs_all, weave=None):
    """k-reduce (packed, strip-interleaved) + exp + row/col sums for tile t.
    `weave` is a list of pending pair-diff steps for the NEXT tile; they are
    interleaved into the PE stream so the next tile's abs pass (ACT/DVE)
    overlaps this tile's k-reduce (PE)."""
    work, ework, pdiff, pnorm = pools
    dve = DVE_TILES[t]
    weave = list(weave or [])
    n_mm = 8 * 16 * (2 if dve else 1)
    stride = max(1, n_mm // (len(weave) + 1)) if weave else 0
    mm_count = 0

    def tick():
        nonlocal mm_count
        mm_count += 1
        if weave and stride and mm_count % stride == 0:
            weave.pop(0)()
    for G in range(2):
        pn = pnorm.tile([128, 512], F32, tag="pn")
        # zero the full tile: cells no matmul writes (j <= i) must read as
        # exact 0 so exp gives exactly 1.0 (host subtracts the known count)
        nc.vector.memset(pn[:], 0.0)
        first = True
        for gl in range(4):
            ig = 4 * G + gl
            for idx in range(16):
                # strip-interleaved: consecutive matmuls hit different
                # 32-col PE strips (q fastest) so they run concurrently
                q, a = idx % 4, idx // 4
                i = 16 * ig + 4 * a + q
                if i >= B - 1:
                    continue
                w = 127 - i
                bs = _pair_base(i)
                out_ap = pn[32 * q:32 * q + 32,
                            128 * gl + i + 1:128 * (gl + 1)]
                last = (gl == 3 and idx == 15)
                nc.tensor.matmul(
                    out_ap, s32_sb[a][:], absd[:, 0, bs:bs + w],
                    start=first, stop=(last and not dve),
                    tile_position=(0, 32 * q), skip_group_check=True)
                first = False
                tick()
                if dve:
                    # second relu plane accumulates into the same columns
                    nc.tensor.matmul(
                        out_ap, s32_sb[a][:], absd[:, 1, bs:bs + w],
                        start=False, stop=last,
                        tile_position=(0, 32 * q), skip_group_check=True)
                    tick()
        e = ework.tile([128, 512], BF16, tag="e")
        nc.scalar.activation(e[:], pn[:], AF.Exp, scale=-2.0)
        # row sums over j within each igroup -> rs_all[:, 8*ig + t]
        rs_view = rs_all.rearrange("p (ig tt) -> p ig tt", tt=8)
        nc.vector.tensor_reduce(
            rs_view[:, 4 * G:4 * G + 4, t],
            e[:].rearrange("p (g j) -> p g j", g=4), op=A.add,
            axis=mybir.AxisListType.X)
        for gl in range(4):
            ig = 4 * G + gl
            nc.tensor.matmul(po[:], s2_sb[t][:],
                             e[:, 128 * gl:128 * (gl + 1)],
                             start=(t == 0 and ig == 0),
                             stop=(t == NT - 1 and ig == 7))
    # flush any unwoven pair-diff steps for the next tile
    for stp in weave:
        stp()


def _build_program():
    nc = bass.Bass()
    xT_d = nc.dram_tensor("xt", [128, NCI, B], FP8, kind="ExternalInput")
    tc_d = nc.dram_tensor("tc", [IN, F], FP8, kind="ExternalInput")
    psel_d = nc.dram_tensor("psel", [B, NPAIR], FP8, kind="ExternalInput")
    s32_d = nc.dram_tensor("s32", [4, 128, 32], BF16, kind="ExternalInput")
    s2_d = nc.dram_tensor("s2", [NT, 128, OC], BF16, kind="ExternalInput")
    po_d = nc.dram_tensor("po", [OC, B], F32, kind="ExternalOutput")
    rs_d = nc.dram_tensor("rs", [128, 64], F32, kind="ExternalOutput")

    with tile.TileContext(nc) as tc:
        with (
            tc.tile_pool(name="cst", bufs=1) as cst,
            tc.tile_pool(name="work", bufs=3) as work,
            tc.tile_pool(name="ework", bufs=4) as ework,
            tc.tile_pool(name="pgemm", bufs=1, space="PSUM") as pgemm,
            tc.tile_pool(name="pdiff", bufs=2, space="PSUM") as pdiff,
            tc.tile_pool(name="pnorm", bufs=2, space="PSUM") as pnorm,
            tc.tile_pool(name="pob", bufs=1, space="PSUM") as pob,
        ):
            # contiguous whole-tensor DMAs: >=1KB per-partition runs.
            # tc gates the GEMM (and thus everything) -> queue it first.
            xt_all = cst.tile([128, NCI, B], FP8, tag="xtall")
            nc.sync.dma_start(xt_all[:], xT_d[:])
            tc_sb = []
            for ci in range(NCI):
                t_ = cst.tile([128, F], FP8, tag=f"tc{ci}")
                nc.sync.dma_start(t_[:], tc_d[128 * ci:128 * (ci + 1), :])
                tc_sb.append(t_)
            xT_sb = [xt_all[:, ci, :] for ci in range(NCI)]
            psel_sb = cst.tile([128, NPAIR], FP8, tag="psel")
            nc.sync.dma_start(psel_sb[:, 0:2048], psel_d[:, 0:2048])
            nc.sync.dma_start(psel_sb[:, 2048:4096], psel_d[:, 2048:4096])
            nc.sync.dma_start(psel_sb[:, 4096:6144], psel_d[:, 4096:6144])
            nc.sync.dma_start(psel_sb[:, 6144:NPAIR], psel_d[:, 6144:NPAIR])
            s32_sb = []
            for a in range(4):
                t_ = cst.tile([128, 32], BF16, tag=f"s32_{a}")
                nc.sync.dma_start(t_[:], s32_d[a])
                s32_sb.append(t_)
            s2_sb = []
            for t in range(NT):
                t_ = cst.tile([128, OC], BF16, tag=f"s2{t}")
                nc.sync.dma_start(t_[:], s2_d[t])
                s2_sb.append(t_)

            # ---- GEMM: m[b, f] = x @ (T_c/2), fp8 operands ----
            m_bf = cst.tile([128, F], FP8, tag="mbf")
            for half in range(2):
                ps = pgemm.tile([128, 512], F32, tag="pg")
                for ci in range(NCI):
                    nc.tensor.matmul(
                        ps[:], xT_sb[ci],
                        tc_sb[ci][:, 512 * half:512 * (half + 1)],
                        start=(ci == 0), stop=(ci == NCI - 1))
                nc.vector.tensor_copy(m_bf[:, 512 * half:512 * (half + 1)],
                                      ps[:])

            po = pob.tile([OC, B], F32, tag="po")
            rs_all = cst.tile([128, 64], F32, tag="rs")

            # software pipeline: tile t's k-reduce (PE) interleaves the
            # pair-diff chunks of tile t+1, so t+1's abs pass (ACT/DVE)
            # overlaps t's k-reduce instead of serializing after it
            pools = (work, ework, pdiff, pnorm)
            absd0, steps0 = _make_pd_abs_steps(nc, pools, 0, m_bf, psel_sb)
            for s in steps0:
                s()
            cur_absd = absd0
            for t in range(NT):
                if t + 1 < NT:
                    nxt_absd, nxt_steps = _make_pd_abs_steps(
                        nc, pools, t + 1, m_bf, psel_sb)
                else:
                    nxt_absd, nxt_steps = None, []
                _emit_kred(nc, pools, t, cur_absd, s32_sb, s2_sb,
                           po, rs_all, weave=nxt_steps)
                cur_absd = nxt_absd

            po_sb = cst.tile([OC, B], F32, tag="posb")
            nc.vector.tensor_copy(po_sb[:], po[:])
            nc.sync.dma_start(po_d[:], po_sb[:])
            nc.sync.dma_start(rs_d[:], rs_all[:])

    _split_excess_waits(nc)
    return nc


def _host_consts():
    psel = np.zeros((B, NPAIR), np.float32)
    col = 0
    for i in range(B - 1):
        w = 127 - i
        psel[i, col:col + w] = 1.0
        psel[np.arange(i + 1, 128), np.arange(col, col + w)] = -1.0
        col += w
    s32 = np.zeros((4, 128, 32), np.float32)
    for a in range(4):
        for osub in range(8):
            s32[a, 16 * osub:16 * (osub + 1), 8 * a + osub] = 1.0
    s2 = np.zeros((NT, 128, OC), np.float32)
    for t in range(NT):
        for p in range(128):
            s2[t, p, 8 * t + (p % 8)] = 1.0
    return (psel.astype(ml_dtypes.float8_e4m3),
            s32.astype(ml_dtypes.bfloat16),
            s2.astype(ml_dtypes.bfloat16))


_CACHE = {}


def _get_cached():
    if "nc" not in _CACHE:
        _CACHE["nc"] = _build_program()
        _CACHE["consts"] = _host_consts()
        # rowsum reindex: rs_all[p, 8*ig + t] belongs to
        # i = 16*ig + 4*a + q with p = 32*q + 8*a + osub, o = 8*t + osub
        p_idx = np.arange(128)
        q, rem = p_idx // 32, p_idx % 32
        a_, osub = rem // 8, rem % 8
        cols = np.arange(64)
        ig, t_ = cols // 8, cols % 8
        i_map = 16 * ig[None, :] + 4 * a_[:, None] + q[:, None]   # [128, 64]
        o_map = 8 * t_[None, :] + osub[:, None]                   # [128, 64]
        _CACHE["i_map"] = i_map
        _CACHE["o_map"] = o_map
    return _CACHE


def kernel(x: np.ndarray, T: np.ndarray, _trace=False, _tmpdir=None) -> np.ndarray:
    x = np.asarray(x, dtype=np.float32)
    T = np.asarray(T, dtype=np.float32)
    c = _get_cached()
    nc = c["nc"]
    psel, s32, s2 = c["consts"]

    xt = np.ascontiguousarray(
        x.T.reshape(NCI, 128, B).transpose(1, 0, 2)
    ).astype(ml_dtypes.float8_e4m3)
    in_maps = []
    for cr in range(NCORES):
        tc_c = np.ascontiguousarray(
            (0.5 * T[:, OC * cr:OC * (cr + 1), :]).reshape(IN, F)
        ).astype(ml_dtypes.float8_e4m3)
        in_maps.append({"xt": xt, "tc": tc_c, "psel": psel,
                        "s32": s32, "s2": s2})

    kw = {}
    if _trace:
        kw = dict(trace=True, tmpdir=_tmpdir)
    res = run_bass_kernel_spmd(nc, in_maps, list(range(NCORES)), **kw)

    jj = np.arange(B, dtype=np.float32)
    junk_col = (B - jj)[None, :]          # po[o, j] junk = 128 - j
    i_map, o_map = c["i_map"], c["o_map"]
    o_b = np.empty((B, OUT), np.float32)
    for cr in range(NCORES):
        r = res.results[cr]
        po = r["po"] - junk_col                       # [64, 128] colsums
        ob_c = po.T.copy()                            # [j, o_local]
        rows = r["rs"] - (i_map + 1)                  # rowsums minus junk
        np.add.at(ob_c, (i_map.ravel(), o_map.ravel()), rows.ravel())
        o_b[:, OC * cr:OC * (cr + 1)] = ob_c
    out = np.concatenate([x, o_b], axis=1)
    if _trace:
        return out, res
    return out



# revision 5
# speedup vs baseline: 1.0425x; 1.0246x over previous
"""MinibatchDiscrimination kernel for 8 Trainium2 NeuronCores.

reference:
    m = einsum('bi,iok->bok', x, T)          # B=128, IN=1024, OUT=512, K=16
    norm[i,j,o] = sum_k |m[j,o,k] - m[i,o,k]|
    o_b = sum_i exp(-norm) - 1               # [B, OUT]
    out = concat([x, o_b], axis=1)           # [128, 1536]

Sharding: each core owns OUT/8 = 64 output features (zero communication).
All device inputs are fp8e4 with T pre-scaled by 1/2 (the exp scale
undoes it); m-quantization error only shifts norms that are >= ~130,
where exp underflows to exactly 0, so the output is bit-exact.

The 8 f-tiles (128 f-rows = 8 o x 16 k each) are split between two
engine pipelines (TILE_KIND):

A-tiles ('A'): pair-diff on PE (m_bf.T @ psel, psel[b,(i,j)] = +1{b==i}
  - 1{b==j} over the 8128 strictly-upper pairs), |.| on ACT in
  [128, 1024] supers, k-reduce on PE per i with packed 32-row strip
  selectors (tile_position). Unwritten PSUM cells (j <= i) read exact 0
  -> exp 1.0, removed host-side via known junk counts.

D-tiles ('D'): no PE pair-diff. DVE computes mm[f, (i', j)] =
  max(m_i, m_j) with one broadcast tensor_tensor op per 16-i super
  (padded row layout), using norm = 2*sum_k max(m_i,m_j) - r_i - r_j.
  The k-reduce contracts the max plane; the -(r_i + r_j)/2 terms are
  added into PSUM by three wide rank matmuls per [128, 512] bank
  (rank-8 o-slot selector for r_j, a fixed permutation matmul over a
  DMA-gathered per-row r column for r_i, and a rank-16 +BIG fill that
  makes every j <= i junk cell underflow to exact 0 in
  exp(-4*P) = exp(-norm)). No host junk correction for D-tiles.

Shared: exp on ACT ([128, 512], scale -2 for A / -4 for D), column
sums via selector matmuls into PSUM [64, 128], row sums via DVE
tensor_reduce, host combines with per-kind junk corrections.
"""

import numpy as np
import ml_dtypes

import concourse.bass as bass
import concourse.tile as tile
from concourse import mybir
from concourse.bass_utils import run_bass_kernel_spmd

BF16 = mybir.dt.bfloat16
F32 = mybir.dt.float32
FP8 = mybir.dt.float8e4
A = mybir.AluOpType
AF = mybir.ActivationFunctionType
DRM = mybir.MatmulPerfMode.DoubleRow

B = 128
IN = 1024
OUT = 512
K = 16
NCORES = 8
OC = OUT // NCORES       # 64
F = OC * K               # 1024
NT = F // 128            # 8 f-tiles
NCI = IN // 128          # 8 contraction chunks
NPAIR = (B * (B - 1)) // 2   # 8128 strictly-upper pairs
CHUNK = 512
NCHUNK = (NPAIR + CHUNK - 1) // CHUNK   # 16 (last = 448)
SUPER = 1024
NSUPER = (NPAIR + SUPER - 1) // SUPER   # 8 (last = 960)

# tile kinds: 'A' = PE pair-diff + ACT abs; 'D' = DVE minmax supers
TILE_KIND = "ADADADAA"
DR_PD = False           # (DoubleRow gave no HW rate gain)

# D-tile supers: igroup-aligned supers of 16 i's, padded row width
DSUP_W = [127 - 16 * s for s in range(8)]
DSUP_OFF = np.cumsum([0] + [16 * w for w in DSUP_W]).tolist()
DW = DSUP_OFF[-1]       # 16*(127+111+...+15) = 9088


def _pair_base(i):
    return i * 127 - (i * (i - 1)) // 2


def _split_excess_waits(nc, max_waits=1):
    """This walrus build rejects instructions carrying more than one sem
    wait; hoist extras onto preceding NoOps on the same engine."""
    for fn in nc.m.functions:
        for blk in fn.blocks:
            new_insts = []
            for inst in blk.instructions:
                si = inst.sync_info
                if si and si.on_wait and len(si.on_wait) > max_waits:
                    waits = list(si.on_wait)
                    extra, keep = waits[:-max_waits], waits[-max_waits:]
                    k = 0
                    while extra:
                        chunk, extra = extra[:max_waits], extra[max_waits:]
                        nop = mybir.InstNoOp(
                            name=f"{inst.name}-ws{k}", engine=inst.engine,
                            ins=[], outs=[],
                            sync_info=mybir.SyncInfo(on_wait=chunk, on_update=[]))
                        nc.register_instruction(nop)
                        new_insts.append(nop)
                        k += 1
                    inst.sync_info = mybir.SyncInfo(
                        on_wait=keep, on_update=list(si.on_update))
                new_insts.append(inst)
            blk.instructions[:] = new_insts


def _make_a_steps(nc, pools, t, m8, psel8, m_bf, psel_sb):
    """A-tile: pair-diff chunks (PE) + |.| (ACT) -> absd bf16."""
    work, ework, pdiff, pnorm = pools
    absd = work.tile([128, NPAIR], BF16, tag="absd")

    def step(c):
        lo = c * SUPER
        w = min(SUPER, NPAIR - lo)
        pd = pdiff.tile([128, SUPER], F32, tag="pd")
        for h in range(0, w, CHUNK):
            hw = min(CHUNK, w - h)
            if DR_PD:
                nc.tensor.matmul(
                    pd[:, h:h + hw], m8[:, :, 128 * t:128 * (t + 1)],
                    psel8[:, :, lo + h:lo + h + hw],
                    start=True, stop=True, perf_mode=DRM,
                    skip_group_check=True)
            else:
                nc.tensor.matmul(
                    pd[:, h:h + hw], m_bf[:, 128 * t:128 * (t + 1)],
                    psel_sb[:, lo + h:lo + h + hw],
                    start=True, stop=True)
        nc.scalar.activation(absd[:, lo:lo + w], pd[:, 0:w], AF.Abs)

    return absd, [lambda c=c: step(c) for c in range(NSUPER)]


def _make_d_steps(nc, pools, t, m_t):
    """D-tile (max form): mm[f, (i', j)] = max(m_i, m_j) on DVE, one
    broadcast op per 16-i super. norm = 2*sum_k max - r_i - r_j; the r
    terms and a +BIG junk fill are added into PSUM by rank matmuls."""
    work, ework, pdiff, pnorm = pools
    mm = work.tile([128, DW], BF16, tag="mm")

    def step(s):
        w = DSUP_W[s]
        off = DSUP_OFF[s]
        in0 = m_t[:, 16 * s:16 * s + 16].unsqueeze(2).broadcast_to(
            [128, 16, w])
        in1 = m_t[:, 16 * s + 1:128].unsqueeze(1).broadcast_to(
            [128, 16, w])
        nc.vector.tensor_tensor(
            mm[:, off:off + 16 * w].rearrange("p (a b) -> p a b", a=16),
            in0, in1, op=A.max)

    return mm, [lambda s=s: step(s) for s in range(8)]


def _emit_kred(nc, pools, t, kind, buf, s32_sb, s32n_sb, s2_sb, po, rs_all,
               weave=None, last_tile=False, raux=None):
    """k-reduce + exp + row/col sums for tile t (both kinds).
    `weave` interleaves the NEXT tile's production steps into the PE
    stream so its ACT/DVE work overlaps this tile's k-reduce."""
    work, ework, pdiff, pnorm = pools
    weave = list(weave or [])
    n_mm = 8 * 16 * (2 if kind == "D" else 1)
    stride = max(1, n_mm // (len(weave) + 1)) if weave else 0
    mm_count = 0

    def tick():
        nonlocal mm_count
        mm_count += 1
        if weave and stride and mm_count % stride == 0:
            weave.pop(0)()

    for G in range(2):
        pn = pnorm.tile([128, 512], F32, tag="pn")
        nc.vector.memset(pn[:], 0.0)
        for gl in range(4):
            ig = 4 * G + gl
            for idx in range(16):
                q, a = idx % 4, idx // 4
                i = 16 * ig + 4 * a + q
                if i >= B - 1:
                    continue
                w = 127 - i
                out_ap = pn[32 * q:32 * q + 32,
                            128 * gl + i + 1:128 * (gl + 1)]
                last = (gl == 3 and idx == 15) and kind == "A"
                if kind == "A":
                    bs = _pair_base(i)
                    nc.tensor.matmul(
                        out_ap, s32_sb[a][:], buf[:, bs:bs + w],
                        start=False, stop=last,
                        tile_position=(0, 32 * q), skip_group_check=True)
                    tick()
                else:
                    s = ig
                    isub = i - 16 * s
                    cs = DSUP_OFF[s] + isub * DSUP_W[s] + (i - 16 * s)
                    nc.tensor.matmul(
                        out_ap, s32_sb[a][:], buf[:, cs:cs + w],
                        start=False, stop=False,
                        tile_position=(0, 32 * q), skip_group_check=True)
                    tick()
        if kind == "D":
            l2o_sb, rtn, perm_sb, rpk2, lf16_sb, rfill_sb = raux
            # r_j: same [8,128] row tile for every gl block (0-stride rep)
            nc.tensor.matmul(
                pn[:], l2o_sb[:],
                rtn[:].unsqueeze(1).broadcast_to([8, 4, 128]),
                start=False, stop=False, skip_group_check=True)
            # r_i: per-igroup gathered column, broadcast along j
            nc.tensor.matmul(
                pn[:], perm_sb[:],
                rpk2[:, 4 * G:4 * G + 4].unsqueeze(2).broadcast_to(
                    [128, 4, 128]),
                start=False, stop=False, skip_group_check=True)
            # +BIG fill on j <= i cells
            nc.tensor.matmul(
                pn[:], lf16_sb[:],
                rfill_sb[:, 512 * G:512 * (G + 1)],
                start=False, stop=True, skip_group_check=True)
        e = ework.tile([128, 512], BF16, tag="e")
        # A-tiles: exp(-2*norm_half). D-tiles: PSUM P = sum_k max
        # - (r_i + r_j)/2 (+BIG on j<=i) and norm_half = 2P -> exp(-4P),
        # junk underflows to exactly 0 (no host correction).
        nc.scalar.activation(e[:], pn[:], AF.Exp,
                             scale=(-4.0 if kind == "D" else -2.0))
        rs_view = rs_all.rearrange("p (ig tt) -> p ig tt", tt=8)
        nc.vector.tensor_reduce(
            rs_view[:, 4 * G:4 * G + 4, t],
            e[:].rearrange("p (g j) -> p g j", g=4), op=A.add,
            axis=mybir.AxisListType.X)
        for gl in range(4):
            ig = 4 * G + gl
            nc.tensor.matmul(po[:], s2_sb[t][:],
                             e[:, 128 * gl:128 * (gl + 1)],
                             start=(t == 0 and ig == 0),
                             stop=(last_tile and ig == 7))
    for stp in weave:
        stp()


def _build_program():
    nc = bass.Bass()
    xT_d = nc.dram_tensor("xt", [IN, B], BF16, kind="ExternalInput")
    tc_d = nc.dram_tensor("tc", [IN, F], BF16, kind="ExternalInput")
    s32_d = nc.dram_tensor("s32", [4, 128, 32], BF16, kind="ExternalInput")
    s8k_d = nc.dram_tensor("s8k", [128, 8], BF16, kind="ExternalInput")
    l2o_d = nc.dram_tensor("l2o", [8, 128], BF16, kind="ExternalInput")
    perm_d = nc.dram_tensor("perm", [128, 128], BF16, kind="ExternalInput")
    lf16_d = nc.dram_tensor("lf16", [16, 128], BF16, kind="ExternalInput")
    rfill_d = nc.dram_tensor("rfill", [16, 8 * 128], BF16,
                             kind="ExternalInput")
    s32n_d = nc.dram_tensor("s32n", [4, 128, 32], BF16, kind="ExternalInput")
    s2_d = nc.dram_tensor("s2", [NT, 128, OC], BF16, kind="ExternalInput")
    po_d = nc.dram_tensor("po", [OC, B], F32, kind="ExternalOutput")
    rs_d = nc.dram_tensor("rs", [128, 64], F32, kind="ExternalOutput")
    xt8f_d = nc.dram_tensor("xt8f", [128, NCI, B], FP8, kind="ExternalInput")
    tc8f_d = nc.dram_tensor("tc8f", [IN, F], FP8, kind="ExternalInput")
    if DR_PD:
        # x arranged for DR GEMM: xt8[cp, p, kt, b] = x.T[256cp+128kt+p, b]
        xt8_d = nc.dram_tensor("xt8", [4, 128, 2, B], FP8,
                               kind="ExternalInput")
        tc8_d = nc.dram_tensor("tc8", [4, 128, 2, F], FP8,
                               kind="ExternalInput")
        psel8_d = nc.dram_tensor("psel8", [64, 2, NPAIR], FP8,
                                 kind="ExternalInput")
    else:
        psel_d = nc.dram_tensor("psel", [B, NPAIR], FP8,
                                kind="ExternalInput")

    a_tiles = [t for t in range(NT) if TILE_KIND[t] == "A"]
    d_tiles = [t for t in range(NT) if TILE_KIND[t] == "D"]

    with tile.TileContext(nc) as tc:
        with (
            tc.tile_pool(name="cst", bufs=1) as cst,
            tc.tile_pool(name="work", bufs=3) as work,
            tc.tile_pool(name="ework", bufs=4) as ework,
            tc.tile_pool(name="pdiff", bufs=2, space="PSUM") as pdiff,
            tc.tile_pool(name="pnorm", bufs=2, space="PSUM") as pnorm,
            tc.tile_pool(name="pob", bufs=1, space="PSUM") as pob,
        ):
            pools = (work, ework, pdiff, pnorm)

            # ---- constant loads ----
            s32_sb, s32n_sb = [], []
            for a in range(4):
                t_ = cst.tile([128, 32], BF16, tag=f"s32_{a}")
                nc.sync.dma_start(t_[:], s32_d[a])
                s32_sb.append(t_)
                t_ = cst.tile([128, 32], BF16, tag=f"s32n_{a}")
                nc.sync.dma_start(t_[:], s32n_d[a])
                s32n_sb.append(t_)
            s2_sb = []
            for t in range(NT):
                t_ = cst.tile([128, OC], BF16, tag=f"s2{t}")
                nc.sync.dma_start(t_[:], s2_d[t])
                s2_sb.append(t_)

            m_bf, psel_sb, m8, psel8 = None, None, None, None
            if DR_PD:
                xt8_sb, tc8_sb = [], []
                for cp in range(4):
                    t_ = cst.tile([128, 2, B], FP8, tag=f"xt8{cp}")
                    nc.sync.dma_start(t_[:], xt8_d[cp])
                    xt8_sb.append(t_)
                    t_ = cst.tile([128, 2, F], FP8, tag=f"tc8{cp}")
                    nc.sync.dma_start(t_[:], tc8_d[cp])
                    tc8_sb.append(t_)
                psel8 = cst.tile([64, 2, NPAIR], FP8, tag="psel8")
                for cch in range(NCHUNK):
                    lo = cch * CHUNK
                    w = min(CHUNK, NPAIR - lo)
                    nc.sync.dma_start(psel8[:, :, lo:lo + w],
                                      psel8_d[:, :, lo:lo + w])
                # ---- DR GEMM -> m8 [64, 2, F] fp8 ----
                m8 = cst.tile([64, 2, F], FP8, tag="m8")
                for half in range(2):
                    for H in range(2):
                        psf = pdiff.tile([128, SUPER], F32, tag="pd")
                        ps = psf[0:64, 0:512]
                        for cp in range(4):
                            nc.tensor.matmul(
                                ps,
                                xt8_sb[cp][:, :, 64 * H:64 * (H + 1)],
                                tc8_sb[cp][:, :, 512 * half:512 * (half + 1)],
                                start=(cp == 0), stop=(cp == 3),
                                perf_mode=DRM, skip_group_check=True)
                        nc.vector.tensor_copy(
                            m8[0:64, H, 512 * half:512 * (half + 1)], ps)
            else:
                xt_all = cst.tile([128, NCI, B], FP8, tag="xtall")
                nc.sync.dma_start(xt_all[:], xt8f_d[:])
                xT_sb = [xt_all[:, ci, :] for ci in range(NCI)]
                tcc_sb = []
                for ci in range(NCI):
                    t_ = cst.tile([128, F], FP8, tag=f"tcc{ci}")
                    nc.sync.dma_start(t_[:], tc8f_d[128 * ci:128 * (ci + 1), :])
                    tcc_sb.append(t_)
                psel_sb = cst.tile([128, NPAIR], FP8, tag="psel")
                nc.sync.dma_start(psel_sb[:, 0:4096], psel_d[:, 0:4096])
                nc.sync.dma_start(psel_sb[:, 4096:NPAIR],
                                  psel_d[:, 4096:NPAIR])
                m_bf = cst.tile([128, F], FP8, tag="mbf")
                for half in range(2):
                    psf = pdiff.tile([128, SUPER], F32, tag="pd")
                    ps = psf[:, 0:512]
                    for ci in range(NCI):
                        nc.tensor.matmul(
                            ps, xT_sb[ci],
                            tcc_sb[ci][:, 512 * half:512 * (half + 1)],
                            start=(ci == 0), stop=(ci == NCI - 1))
                    nc.vector.tensor_copy(
                        m_bf[:, 512 * half:512 * (half + 1)], ps)

            # ---- transposed GEMM for D-tiles: m_t [f, b] bf16 ----
            if d_tiles:
                s8k_sb = cst.tile([128, 8], BF16, tag="s8k")
                nc.sync.dma_start(s8k_sb[:], s8k_d[:])
                l2o_sb = cst.tile([8, 128], BF16, tag="l2o")
                nc.sync.dma_start(l2o_sb[:], l2o_d[:])
                perm_sb = cst.tile([128, 128], BF16, tag="perm")
                nc.sync.dma_start(perm_sb[:], perm_d[:])
                lf16_sb = cst.tile([16, 128], BF16, tag="lf16")
                nc.sync.dma_start(lf16_sb[:], lf16_d[:])
                rfill_sb = cst.tile([16, 8 * 128], BF16, tag="rfill")
                nc.sync.dma_start(rfill_sb[:], rfill_d[:])
                m_T, r_aux = {}, {}
                for t in d_tiles:
                    pmf = pdiff.tile([128, SUPER], F32, tag="pd")
                    pm = pmf[:, 0:128]
                    for ci in range(NCI):
                        nc.tensor.matmul(
                            pm, tcc_sb[ci][:, 128 * t:128 * (t + 1)],
                            xT_sb[ci],
                            start=(ci == 0), stop=(ci == NCI - 1))
                    mt = cst.tile([128, 128], BF16, tag=f"mt{t}")
                    nc.scalar.activation(mt[:], pm, AF.Copy, scale=1.0)
                    m_T[t] = mt
                    # rtn[osub, i] = -0.5 * sum_k m_t[(osub,k), i]
                    prf = pdiff.tile([128, SUPER], F32, tag="pd")
                    pr = prf[0:8, 0:128]
                    nc.tensor.matmul(pr, s8k_sb[:], mt[:],
                                     start=True, stop=True,
                                     skip_group_check=True)
                    rtn = cst.tile([8, 128], BF16, tag=f"rtn{t}")
                    nc.vector.tensor_scalar(rtn[:], pr, -0.5, None,
                                            op0=A.mult)
                    # rpk2[c= (osub,a,q)-major, ig] = rtn[osub, 16ig+4a+q]
                    rpk2 = cst.tile([128, 8], BF16, tag=f"rpk{t}")
                    for ig in range(8):
                        nc.sync.dma_start(
                            rpk2[:, ig:ig + 1],
                            rtn[:, 16 * ig:16 * ig + 16].unsqueeze(2))
                    r_aux[t] = (rtn, rpk2)

            po = pob.tile([OC, B], F32, tag="po")
            rs_all = cst.tile([128, 64], F32, tag="rs")

            # ---- software pipeline over tiles ----
            def make_steps(t):
                if TILE_KIND[t] == "A":
                    return _make_a_steps(nc, pools, t, m8, psel8,
                                         m_bf, psel_sb)
                return _make_d_steps(nc, pools, t, m_T[t])

            cur_buf, steps0 = make_steps(0)
            for s in steps0:
                s()
            for t in range(NT):
                if t + 1 < NT:
                    nxt_buf, nxt_steps = make_steps(t + 1)
                else:
                    nxt_buf, nxt_steps = None, []
                if TILE_KIND[t] == "D":
                    rtn, rpk2 = r_aux[t]
                    raux = (l2o_sb, rtn, perm_sb, rpk2, lf16_sb, rfill_sb)
                else:
                    raux = None
                _emit_kred(nc, pools, t, TILE_KIND[t], cur_buf,
                           s32_sb, s32n_sb, s2_sb, po, rs_all,
                           weave=nxt_steps, last_tile=(t == NT - 1),
                           raux=raux)
                cur_buf = nxt_buf

            po_sb = cst.tile([OC, B], F32, tag="posb")
            nc.vector.tensor_copy(po_sb[:], po[:])
            nc.sync.dma_start(po_d[:], po_sb[:])
            nc.sync.dma_start(rs_d[:], rs_all[:])

    _split_excess_waits(nc)
    return nc


def _host_consts():
    psel = np.zeros((B, NPAIR), np.float32)
    col = 0
    for i in range(B - 1):
        w = 127 - i
        psel[i, col:col + w] = 1.0
        psel[np.arange(i + 1, 128), np.arange(col, col + w)] = -1.0
        col += w
    s32 = np.zeros((4, 128, 32), np.float32)
    for a in range(4):
        for osub in range(8):
            s32[a, 16 * osub:16 * (osub + 1), 8 * a + osub] = 1.0
    s2 = np.zeros((NT, 128, OC), np.float32)
    for t in range(NT):
        for p in range(128):
            s2[t, p, 8 * t + (p % 8)] = 1.0
    s8k = np.zeros((128, 8), np.float32)
    for f in range(128):
        s8k[f, f // 16] = 1.0
    l2o = np.zeros((8, 128), np.float32)
    for p in range(128):
        l2o[p % 8, p] = 1.0
    perm = np.zeros((128, 128), np.float32)
    for cc in range(128):
        osub, rem = cc // 16, cc % 16
        a_, q_ = rem // 4, rem % 4
        perm[cc, 32 * q_ + 8 * a_ + osub] = 1.0
    BIG = 30000.0
    lf16 = np.zeros((16, 128), np.float32)
    for p in range(128):
        q_, rem = p // 32, p % 32
        a_ = rem // 8
        lf16[4 * a_ + q_, p] = 1.0
    rfill = np.zeros((16, 8 * 128), np.float32)
    for ig in range(8):
        for sp in range(16):
            a_, q_ = sp // 4, sp % 4
            i = 16 * ig + 4 * a_ + q_
            rfill[sp, 128 * ig:128 * ig + i + 1] = BIG
    bf = ml_dtypes.bfloat16
    return (psel, s32.astype(bf), s2.astype(bf),
            s8k.astype(bf), l2o.astype(bf), perm.astype(bf),
            lf16.astype(bf), rfill.astype(bf))


_CACHE = {}


def _get_cached():
    if "nc" not in _CACHE:
        _CACHE["nc"] = _build_program()
        _CACHE["consts"] = _host_consts()
        p_idx = np.arange(128)
        q, rem = p_idx // 32, p_idx % 32
        a_, osub = rem // 8, rem % 8
        cols = np.arange(64)
        ig, t_ = cols // 8, cols % 8
        i_map = 16 * ig[None, :] + 4 * a_[:, None] + q[:, None]   # [128, 64]
        o_map = 8 * t_[None, :] + osub[:, None]                   # [128, 64]
        _CACHE["i_map"] = i_map
        _CACHE["o_map"] = o_map
    return _CACHE


def kernel(x: np.ndarray, T: np.ndarray, _trace=False, _tmpdir=None) -> np.ndarray:
    x = np.asarray(x, dtype=np.float32)
    T = np.asarray(T, dtype=np.float32)
    c = _get_cached()
    nc = c["nc"]
    (psel, s32, s2, s8k, l2o, perm, lf16, rfill) = c["consts"]
    psel = np.asarray(psel, np.float32)
    s32n = (-np.asarray(s32, np.float32)).astype(ml_dtypes.bfloat16)

    xt = np.ascontiguousarray(x.T)
    in_maps = []
    for cr in range(NCORES):
        # T scaled by 1/2 so m fits fp8e4 and diffs stay in range
        tc_c = np.ascontiguousarray(
            (0.5 * T[:, OC * cr:OC * (cr + 1), :]).reshape(IN, F))
        im = {"xt": xt.astype(ml_dtypes.bfloat16),
              "tc": tc_c.astype(ml_dtypes.bfloat16),
              "s32": s32, "s32n": s32n, "s2": s2,
              "s8k": s8k, "l2o": l2o, "perm": perm, "lf16": lf16,
              "rfill": rfill}
        if DR_PD:
            im["xt8"] = np.ascontiguousarray(
                xt.reshape(4, 2, 128, B).transpose(0, 2, 1, 3)
            ).astype(ml_dtypes.float8_e4m3)
            im["tc8"] = np.ascontiguousarray(
                tc_c.reshape(4, 2, 128, F).transpose(0, 2, 1, 3)
            ).astype(ml_dtypes.float8_e4m3)
            im["psel8"] = np.ascontiguousarray(
                psel.reshape(2, 64, NPAIR).transpose(1, 0, 2)
            ).astype(ml_dtypes.float8_e4m3)
        else:
            im["psel"] = psel.astype(ml_dtypes.float8_e4m3)
            im["xt8f"] = np.ascontiguousarray(
                xt.reshape(NCI, 128, B).transpose(1, 0, 2)
            ).astype(ml_dtypes.float8_e4m3)
            im["tc8f"] = tc_c.astype(ml_dtypes.float8_e4m3)
        in_maps.append(im)

    kw = {}
    if _trace:
        kw = dict(trace=True, tmpdir=_tmpdir)
    res = run_bass_kernel_spmd(nc, in_maps, list(range(NCORES)), **kw)

    jj = np.arange(B, dtype=np.float32)
    junk_col = (B - jj)[None, :]          # A-tile po junk = 128 - j
    i_map, o_map = c["i_map"], c["o_map"]
    # junk corrections apply only to A-tile rows/cols (D junk is exact 0)
    a_mask_o = np.zeros((OC, 1), np.float32)   # po rows: o = 8t + osub
    for t in range(NT):
        if TILE_KIND[t] == "A":
            a_mask_o[8 * t:8 * t + 8] = 1.0
    a_mask_rs = np.zeros((1, 64), np.float32)  # rs cols: c = 8ig + t
    for col in range(64):
        if TILE_KIND[col % 8] == "A":
            a_mask_rs[0, col] = 1.0
    o_b = np.empty((B, OUT), np.float32)
    for cr in range(NCORES):
        r = res.results[cr]
        po = r["po"] - junk_col * a_mask_o            # [64, 128] colsums
        ob_c = po.T.copy()                            # [j, o_local]
        rows = r["rs"] - (i_map + 1) * a_mask_rs      # rowsums minus junk
        np.add.at(ob_c, (i_map.ravel(), o_map.ravel()), rows.ravel())
        o_b[:, OC * cr:OC * (cr + 1)] = ob_c
    out = np.concatenate([x, o_b], axis=1)
    if _trace:
        return out, res
    return out


# revision 6
# speedup vs baseline: 1.0681x; 1.0246x over previous
"""MinibatchDiscrimination kernel for 8 Trainium2 NeuronCores.

reference:
    m = einsum('bi,iok->bok', x, T)          # B=128, IN=1024, OUT=512, K=16
    norm[i,j,o] = sum_k |m[j,o,k] - m[i,o,k]|
    o_b = sum_i exp(-norm) - 1               # [B, OUT]
    out = concat([x, o_b], axis=1)           # [128, 1536]

Sharding: each core owns OUT/8 = 64 output features (zero communication).
All device inputs are fp8e4 with T pre-scaled by 1/2 (the exp scale
undoes it); m-quantization error only shifts norms that are >= ~130,
where exp underflows to exactly 0, so the output is bit-exact.

The 8 f-tiles (128 f-rows = 8 o x 16 k each) are split between two
engine pipelines (TILE_KIND):

A-tiles ('A'): pair-diff on PE (m_bf.T @ psel, psel[b,(i,j)] = +1{b==i}
  - 1{b==j} over the 8128 strictly-upper pairs), |.| on ACT in
  [128, 1024] supers, k-reduce on PE per i with packed 32-row strip
  selectors (tile_position). Unwritten PSUM cells (j <= i) read exact 0
  -> exp 1.0, removed host-side via known junk counts.

D-tiles ('D'): no PE pair-diff. DVE computes mm[f, (i', j)] =
  max(m_i, m_j) with one broadcast tensor_tensor op per 16-i super
  (padded row layout), using norm = 2*sum_k max(m_i,m_j) - r_i - r_j.
  The k-reduce contracts the max plane; the -(r_i + r_j)/2 terms are
  added into PSUM by three wide rank matmuls per [128, 512] bank
  (rank-8 o-slot selector for r_j, a fixed permutation matmul over a
  DMA-gathered per-row r column for r_i, and a rank-16 +BIG fill that
  makes every j <= i junk cell underflow to exact 0 in
  exp(-4*P) = exp(-norm)). No host junk correction for D-tiles.

Shared: exp on ACT ([128, 512], scale -2 for A / -4 for D), column
sums via selector matmuls into PSUM [64, 128], row sums via DVE
tensor_reduce, host combines with per-kind junk corrections.
"""

import numpy as np
import ml_dtypes

import concourse.bass as bass
import concourse.tile as tile
from concourse import mybir
from concourse.bass_utils import run_bass_kernel_spmd

BF16 = mybir.dt.bfloat16
F32 = mybir.dt.float32
FP8 = mybir.dt.float8e4
A = mybir.AluOpType
AF = mybir.ActivationFunctionType
DRM = mybir.MatmulPerfMode.DoubleRow

B = 128
IN = 1024
OUT = 512
K = 16
NCORES = 8
OC = OUT // NCORES       # 64
F = OC * K               # 1024
NT = F // 128            # 8 f-tiles
NCI = IN // 128          # 8 contraction chunks
NPAIR = (B * (B - 1)) // 2   # 8128 strictly-upper pairs
CHUNK = 512
NCHUNK = (NPAIR + CHUNK - 1) // CHUNK   # 16 (last = 448)
SUPER = 1024
NSUPER = (NPAIR + SUPER - 1) // SUPER   # 8 (last = 960)

# tile kinds: 'A' = PE pair-diff + ACT abs; 'D' = DVE minmax supers
TILE_KIND = "ADADADAA"
DR_PD = False           # (DoubleRow gave no HW rate gain)

# D-tile supers: igroup-aligned supers of 16 i's, padded row width
DSUP_W = [127 - 16 * s for s in range(8)]
DSUP_OFF = np.cumsum([0] + [16 * w for w in DSUP_W]).tolist()
DW = DSUP_OFF[-1]       # 16*(127+111+...+15) = 9088


def _pair_base(i):
    return i * 127 - (i * (i - 1)) // 2


def _split_excess_waits(nc, max_waits=1):
    """This walrus build rejects instructions carrying more than one sem
    wait; hoist extras onto preceding NoOps on the same engine."""
    for fn in nc.m.functions:
        for blk in fn.blocks:
            new_insts = []
            for inst in blk.instructions:
                si = inst.sync_info
                if si and si.on_wait and len(si.on_wait) > max_waits:
                    waits = list(si.on_wait)
                    extra, keep = waits[:-max_waits], waits[-max_waits:]
                    k = 0
                    while extra:
                        chunk, extra = extra[:max_waits], extra[max_waits:]
                        nop = mybir.InstNoOp(
                            name=f"{inst.name}-ws{k}", engine=inst.engine,
                            ins=[], outs=[],
                            sync_info=mybir.SyncInfo(on_wait=chunk, on_update=[]))
                        nc.register_instruction(nop)
                        new_insts.append(nop)
                        k += 1
                    inst.sync_info = mybir.SyncInfo(
                        on_wait=keep, on_update=list(si.on_update))
                new_insts.append(inst)
            blk.instructions[:] = new_insts


def _make_a_steps(nc, pools, t, m8, psel8, m_bf, psel_sb):
    """A-tile: pair-diff chunks (PE) + |.| (ACT) -> absd bf16."""
    work, ework, pdiff, pnorm = pools
    absd = work.tile([128, NPAIR], BF16, tag="absd")

    def step(c):
        lo = c * SUPER
        w = min(SUPER, NPAIR - lo)
        pd = pdiff.tile([128, SUPER], F32, tag="pd")
        for h in range(0, w, CHUNK):
            hw = min(CHUNK, w - h)
            if DR_PD:
                nc.tensor.matmul(
                    pd[:, h:h + hw], m8[:, :, 128 * t:128 * (t + 1)],
                    psel8[:, :, lo + h:lo + h + hw],
                    start=True, stop=True, perf_mode=DRM,
                    skip_group_check=True)
            else:
                nc.tensor.matmul(
                    pd[:, h:h + hw], m_bf[:, 128 * t:128 * (t + 1)],
                    psel_sb[:, lo + h:lo + h + hw],
                    start=True, stop=True)
        nc.scalar.activation(absd[:, lo:lo + w], pd[:, 0:w], AF.Abs)

    return absd, [lambda c=c: step(c) for c in range(NSUPER)]


def _make_d_steps(nc, pools, t, m_t):
    """D-tile (max form): mm[f, (i', j)] = max(m_i, m_j) on DVE, one
    broadcast op per 16-i super. norm = 2*sum_k max - r_i - r_j; the r
    terms and a +BIG junk fill are added into PSUM by rank matmuls."""
    work, ework, pdiff, pnorm = pools
    mm = work.tile([128, DW], BF16, tag="mm")

    def step(s):
        w = DSUP_W[s]
        off = DSUP_OFF[s]
        in0 = m_t[:, 16 * s:16 * s + 16].unsqueeze(2).broadcast_to(
            [128, 16, w])
        in1 = m_t[:, 16 * s + 1:128].unsqueeze(1).broadcast_to(
            [128, 16, w])
        nc.vector.tensor_tensor(
            mm[:, off:off + 16 * w].rearrange("p (a b) -> p a b", a=16),
            in0, in1, op=A.max)

    return mm, [lambda s=s: step(s) for s in range(8)]


def _emit_kred(nc, pools, t, kind, buf, s32_sb, s32n_sb, s2_sb, po, rs_all,
               weave=None, last_tile=False, raux=None):
    """k-reduce + exp + row/col sums for tile t (both kinds).
    `weave` interleaves the NEXT tile's production steps into the PE
    stream so its ACT/DVE work overlaps this tile's k-reduce."""
    work, ework, pdiff, pnorm = pools
    weave = list(weave or [])
    n_mm = 8 * 16 * (2 if kind == "D" else 1)
    stride = max(1, n_mm // (len(weave) + 1)) if weave else 0
    mm_count = 0

    def tick():
        nonlocal mm_count
        mm_count += 1
        if weave and stride and mm_count % stride == 0:
            weave.pop(0)()

    for G in range(2):
        pn = pnorm.tile([128, 512], F32, tag="pn")
        nc.vector.memset(pn[:], 0.0)
        for gl in range(4):
            ig = 4 * G + gl
            for idx in range(16):
                q, a = idx % 4, idx // 4
                i = 16 * ig + 4 * a + q
                if i >= B - 1:
                    continue
                w = 127 - i
                out_ap = pn[32 * q:32 * q + 32,
                            128 * gl + i + 1:128 * (gl + 1)]
                last = (gl == 3 and idx == 15) and kind == "A"
                if kind == "A":
                    bs = _pair_base(i)
                    nc.tensor.matmul(
                        out_ap, s32_sb[a][:], buf[:, bs:bs + w],
                        start=False, stop=last,
                        tile_position=(0, 32 * q), skip_group_check=True)
                    tick()
                else:
                    s = ig
                    isub = i - 16 * s
                    cs = DSUP_OFF[s] + isub * DSUP_W[s] + (i - 16 * s)
                    nc.tensor.matmul(
                        out_ap, s32_sb[a][:], buf[:, cs:cs + w],
                        start=False, stop=False,
                        tile_position=(0, 32 * q), skip_group_check=True)
                    tick()
        if kind == "D":
            l2o_sb, rtn, perm_sb, rpk2, lf16_sb, rfill_sb = raux
            # r_j: same [8,128] row tile for every gl block (0-stride rep)
            nc.tensor.matmul(
                pn[:], l2o_sb[:],
                rtn[:].unsqueeze(1).broadcast_to([8, 4, 128]),
                start=False, stop=False, skip_group_check=True)
            # r_i: per-igroup gathered column, broadcast along j
            nc.tensor.matmul(
                pn[:], perm_sb[:],
                rpk2[:, 4 * G:4 * G + 4].unsqueeze(2).broadcast_to(
                    [128, 4, 128]),
                start=False, stop=False, skip_group_check=True)
            # +BIG fill on j <= i cells
            nc.tensor.matmul(
                pn[:], lf16_sb[:],
                rfill_sb[:, 512 * G:512 * (G + 1)],
                start=False, stop=True, skip_group_check=True)
        e = ework.tile([128, 512], BF16, tag="e")
        # A-tiles: exp(-2*norm_half). D-tiles: PSUM P = sum_k max
        # - (r_i + r_j)/2 (+BIG on j<=i) and norm_half = 2P -> exp(-4P),
        # junk underflows to exactly 0 (no host correction).
        nc.scalar.activation(e[:], pn[:], AF.Exp,
                             scale=(-4.0 if kind == "D" else -2.0))
        rs_view = rs_all.rearrange("p (ig tt) -> p ig tt", tt=8)
        nc.vector.tensor_reduce(
            rs_view[:, 4 * G:4 * G + 4, t],
            e[:].rearrange("p (g j) -> p g j", g=4), op=A.add,
            axis=mybir.AxisListType.X)
        for gl in range(4):
            ig = 4 * G + gl
            nc.tensor.matmul(po[:], s2_sb[t][:],
                             e[:, 128 * gl:128 * (gl + 1)],
                             start=(t == 0 and ig == 0),
                             stop=(last_tile and ig == 7))
    for stp in weave:
        stp()


def _build_program():
    nc = bass.Bass()
    xT_d = nc.dram_tensor("xt", [IN, B], BF16, kind="ExternalInput")
    tc_d = nc.dram_tensor("tc", [IN, F], BF16, kind="ExternalInput")
    s32_d = nc.dram_tensor("s32", [4, 128, 32], BF16, kind="ExternalInput")
    s8k_d = nc.dram_tensor("s8k", [128, 8], BF16, kind="ExternalInput")
    l2o_d = nc.dram_tensor("l2o", [8, 128], BF16, kind="ExternalInput")
    perm_d = nc.dram_tensor("perm", [128, 128], BF16, kind="ExternalInput")
    lf16_d = nc.dram_tensor("lf16", [16, 128], BF16, kind="ExternalInput")
    rfill_d = nc.dram_tensor("rfill", [16, 8 * 128], BF16,
                             kind="ExternalInput")
    s32n_d = nc.dram_tensor("s32n", [4, 128, 32], BF16, kind="ExternalInput")
    s2_d = nc.dram_tensor("s2", [NT, 128, OC], BF16, kind="ExternalInput")
    po_d = nc.dram_tensor("po", [OC, B], F32, kind="ExternalOutput")
    rs_d = nc.dram_tensor("rs", [128, 64], F32, kind="ExternalOutput")
    xt8f_d = nc.dram_tensor("xt8f", [128, NCI, B], FP8, kind="ExternalInput")
    tc8f_d = nc.dram_tensor("tc8f", [IN, F], FP8, kind="ExternalInput")
    if DR_PD:
        # x arranged for DR GEMM: xt8[cp, p, kt, b] = x.T[256cp+128kt+p, b]
        xt8_d = nc.dram_tensor("xt8", [4, 128, 2, B], FP8,
                               kind="ExternalInput")
        tc8_d = nc.dram_tensor("tc8", [4, 128, 2, F], FP8,
                               kind="ExternalInput")
        psel8_d = nc.dram_tensor("psel8", [64, 2, NPAIR], FP8,
                                 kind="ExternalInput")
    else:
        psel_d = nc.dram_tensor("psel", [B, NPAIR], FP8,
                                kind="ExternalInput")

    a_tiles = [t for t in range(NT) if TILE_KIND[t] == "A"]
    d_tiles = [t for t in range(NT) if TILE_KIND[t] == "D"]

    with tile.TileContext(nc) as tc:
        with (
            tc.tile_pool(name="cst", bufs=1) as cst,
            tc.tile_pool(name="work", bufs=3) as work,
            tc.tile_pool(name="ework", bufs=4) as ework,
            tc.tile_pool(name="pdiff", bufs=2, space="PSUM") as pdiff,
            tc.tile_pool(name="pnorm", bufs=3, space="PSUM") as pnorm,
            tc.tile_pool(name="pob", bufs=1, space="PSUM") as pob,
        ):
            pools = (work, ework, pdiff, pnorm)

            # ---- constant loads ----
            s32_sb, s32n_sb = [], []
            for a in range(4):
                t_ = cst.tile([128, 32], BF16, tag=f"s32_{a}")
                nc.sync.dma_start(t_[:], s32_d[a])
                s32_sb.append(t_)
                t_ = cst.tile([128, 32], BF16, tag=f"s32n_{a}")
                nc.sync.dma_start(t_[:], s32n_d[a])
                s32n_sb.append(t_)
            s2_sb = []
            for t in range(NT):
                t_ = cst.tile([128, OC], BF16, tag=f"s2{t}")
                nc.sync.dma_start(t_[:], s2_d[t])
                s2_sb.append(t_)

            m_bf, psel_sb, m8, psel8 = None, None, None, None
            if DR_PD:
                xt8_sb, tc8_sb = [], []
                for cp in range(4):
                    t_ = cst.tile([128, 2, B], FP8, tag=f"xt8{cp}")
                    nc.sync.dma_start(t_[:], xt8_d[cp])
                    xt8_sb.append(t_)
                    t_ = cst.tile([128, 2, F], FP8, tag=f"tc8{cp}")
                    nc.sync.dma_start(t_[:], tc8_d[cp])
                    tc8_sb.append(t_)
                psel8 = cst.tile([64, 2, NPAIR], FP8, tag="psel8")
                for cch in range(NCHUNK):
                    lo = cch * CHUNK
                    w = min(CHUNK, NPAIR - lo)
                    nc.sync.dma_start(psel8[:, :, lo:lo + w],
                                      psel8_d[:, :, lo:lo + w])
                # ---- DR GEMM -> m8 [64, 2, F] fp8 ----
                m8 = cst.tile([64, 2, F], FP8, tag="m8")
                for half in range(2):
                    for H in range(2):
                        psf = pdiff.tile([128, SUPER], F32, tag="pd")
                        ps = psf[0:64, 0:512]
                        for cp in range(4):
                            nc.tensor.matmul(
                                ps,
                                xt8_sb[cp][:, :, 64 * H:64 * (H + 1)],
                                tc8_sb[cp][:, :, 512 * half:512 * (half + 1)],
                                start=(cp == 0), stop=(cp == 3),
                                perf_mode=DRM, skip_group_check=True)
                        nc.vector.tensor_copy(
                            m8[0:64, H, 512 * half:512 * (half + 1)], ps)
            else:
                xt_all = cst.tile([128, NCI, B], FP8, tag="xtall")
                nc.sync.dma_start(xt_all[:], xt8f_d[:])
                xT_sb = [xt_all[:, ci, :] for ci in range(NCI)]
                tcc_sb = []
                for ci in range(NCI):
                    t_ = cst.tile([128, F], FP8, tag=f"tcc{ci}")
                    nc.sync.dma_start(t_[:], tc8f_d[128 * ci:128 * (ci + 1), :])
                    tcc_sb.append(t_)
                psel_sb = cst.tile([128, NPAIR], FP8, tag="psel")
                nc.sync.dma_start(psel_sb[:, 0:4096], psel_d[:, 0:4096])
                nc.sync.dma_start(psel_sb[:, 4096:NPAIR],
                                  psel_d[:, 4096:NPAIR])
                m_bf = cst.tile([128, F], FP8, tag="mbf")
                for half in range(2):
                    psf = pdiff.tile([128, SUPER], F32, tag="pd")
                    ps = psf[:, 0:512]
                    for ci in range(NCI):
                        nc.tensor.matmul(
                            ps, xT_sb[ci],
                            tcc_sb[ci][:, 512 * half:512 * (half + 1)],
                            start=(ci == 0), stop=(ci == NCI - 1))
                    nc.vector.tensor_copy(
                        m_bf[:, 512 * half:512 * (half + 1)], ps)

            # ---- transposed GEMM for D-tiles: m_t [f, b] bf16 ----
            if d_tiles:
                s8k_sb = cst.tile([128, 8], BF16, tag="s8k")
                nc.sync.dma_start(s8k_sb[:], s8k_d[:])
                l2o_sb = cst.tile([8, 128], BF16, tag="l2o")
                nc.sync.dma_start(l2o_sb[:], l2o_d[:])
                perm_sb = cst.tile([128, 128], BF16, tag="perm")
                nc.sync.dma_start(perm_sb[:], perm_d[:])
                lf16_sb = cst.tile([16, 128], BF16, tag="lf16")
                nc.sync.dma_start(lf16_sb[:], lf16_d[:])
                rfill_sb = cst.tile([16, 8 * 128], BF16, tag="rfill")
                nc.sync.dma_start(rfill_sb[:], rfill_d[:])
                m_T, r_aux = {}, {}
                for t in d_tiles:
                    pmf = pdiff.tile([128, SUPER], F32, tag="pd")
                    pm = pmf[:, 0:128]
                    for ci in range(NCI):
                        nc.tensor.matmul(
                            pm, tcc_sb[ci][:, 128 * t:128 * (t + 1)],
                            xT_sb[ci],
                            start=(ci == 0), stop=(ci == NCI - 1))
                    mt = cst.tile([128, 128], BF16, tag=f"mt{t}")
                    nc.scalar.activation(mt[:], pm, AF.Copy, scale=1.0)
                    m_T[t] = mt
                    # rtn[osub, i] = -0.5 * sum_k m_t[(osub,k), i]
                    prf = pdiff.tile([128, SUPER], F32, tag="pd")
                    pr = prf[0:8, 0:128]
                    nc.tensor.matmul(pr, s8k_sb[:], mt[:],
                                     start=True, stop=True,
                                     skip_group_check=True)
                    rtn = cst.tile([8, 128], BF16, tag=f"rtn{t}")
                    nc.vector.tensor_scalar(rtn[:], pr, -0.5, None,
                                            op0=A.mult)
                    # rpk2[c= (osub,a,q)-major, ig] = rtn[osub, 16ig+4a+q]
                    rpk2 = cst.tile([128, 8], BF16, tag=f"rpk{t}")
                    for ig in range(8):
                        nc.sync.dma_start(
                            rpk2[:, ig:ig + 1],
                            rtn[:, 16 * ig:16 * ig + 16].unsqueeze(2))
                    r_aux[t] = (rtn, rpk2)

            po = pob.tile([OC, B], F32, tag="po")
            rs_all = cst.tile([128, 64], F32, tag="rs")

            # ---- software pipeline over tiles ----
            def make_steps(t):
                if TILE_KIND[t] == "A":
                    return _make_a_steps(nc, pools, t, m8, psel8,
                                         m_bf, psel_sb)
                return _make_d_steps(nc, pools, t, m_T[t])

            cur_buf, steps0 = make_steps(0)
            for s in steps0:
                s()
            for t in range(NT):
                if t + 1 < NT:
                    nxt_buf, nxt_steps = make_steps(t + 1)
                else:
                    nxt_buf, nxt_steps = None, []
                if TILE_KIND[t] == "D":
                    rtn, rpk2 = r_aux[t]
                    raux = (l2o_sb, rtn, perm_sb, rpk2, lf16_sb, rfill_sb)
                else:
                    raux = None
                _emit_kred(nc, pools, t, TILE_KIND[t], cur_buf,
                           s32_sb, s32n_sb, s2_sb, po, rs_all,
                           weave=nxt_steps, last_tile=(t == NT - 1),
                           raux=raux)
                cur_buf = nxt_buf

            po_sb = cst.tile([OC, B], F32, tag="posb")
            nc.vector.tensor_copy(po_sb[:], po[:])
            nc.sync.dma_start(po_d[:], po_sb[:])
            nc.sync.dma_start(rs_d[:], rs_all[:])

    _split_excess_waits(nc)
    return nc


def _host_consts():
    psel = np.zeros((B, NPAIR), np.float32)
    col = 0
    for i in range(B - 1):
        w = 127 - i
        psel[i, col:col + w] = 1.0
        psel[np.arange(i + 1, 128), np.arange(col, col + w)] = -1.0
        col += w
    s32 = np.zeros((4, 128, 32), np.float32)
    for a in range(4):
        for osub in range(8):
            s32[a, 16 * osub:16 * (osub + 1), 8 * a + osub] = 1.0
    s2 = np.zeros((NT, 128, OC), np.float32)
    for t in range(NT):
        for p in range(128):
            s2[t, p, 8 * t + (p % 8)] = 1.0
    s8k = np.zeros((128, 8), np.float32)
    for f in range(128):
        s8k[f, f // 16] = 1.0
    l2o = np.zeros((8, 128), np.float32)
    for p in range(128):
        l2o[p % 8, p] = 1.0
    perm = np.zeros((128, 128), np.float32)
    for cc in range(128):
        osub, rem = cc // 16, cc % 16
        a_, q_ = rem // 4, rem % 4
        perm[cc, 32 * q_ + 8 * a_ + osub] = 1.0
    BIG = 30000.0
    lf16 = np.zeros((16, 128), np.float32)
    for p in range(128):
        q_, rem = p // 32, p % 32
        a_ = rem // 8
        lf16[4 * a_ + q_, p] = 1.0
    rfill = np.zeros((16, 8 * 128), np.float32)
    for ig in range(8):
        for sp in range(16):
            a_, q_ = sp // 4, sp % 4
            i = 16 * ig + 4 * a_ + q_
            rfill[sp, 128 * ig:128 * ig + i + 1] = BIG
    bf = ml_dtypes.bfloat16
    return (psel, s32.astype(bf), s2.astype(bf),
            s8k.astype(bf), l2o.astype(bf), perm.astype(bf),
            lf16.astype(bf), rfill.astype(bf))


_CACHE = {}


def _get_cached():
    if "nc" not in _CACHE:
        _CACHE["nc"] = _build_program()
        _CACHE["consts"] = _host_consts()
        p_idx = np.arange(128)
        q, rem = p_idx // 32, p_idx % 32
        a_, osub = rem // 8, rem % 8
        cols = np.arange(64)
        ig, t_ = cols // 8, cols % 8
        i_map = 16 * ig[None, :] + 4 * a_[:, None] + q[:, None]   # [128, 64]
        o_map = 8 * t_[None, :] + osub[:, None]                   # [128, 64]
        _CACHE["i_map"] = i_map
        _CACHE["o_map"] = o_map
    return _CACHE


def kernel(x: np.ndarray, T: np.ndarray, _trace=False, _tmpdir=None) -> np.ndarray:
    x = np.asarray(x, dtype=np.float32)
    T = np.asarray(T, dtype=np.float32)
    c = _get_cached()
    nc = c["nc"]
    (psel, s32, s2, s8k, l2o, perm, lf16, rfill) = c["consts"]
    psel = np.asarray(psel, np.float32)
    s32n = (-np.asarray(s32, np.float32)).astype(ml_dtypes.bfloat16)

    xt = np.ascontiguousarray(x.T)
    in_maps = []
    for cr in range(NCORES):
        # T scaled by 1/2 so m fits fp8e4 and diffs stay in range
        tc_c = np.ascontiguousarray(
            (0.5 * T[:, OC * cr:OC * (cr + 1), :]).reshape(IN, F))
        im = {"xt": xt.astype(ml_dtypes.bfloat16),
              "tc": tc_c.astype(ml_dtypes.bfloat16),
              "s32": s32, "s32n": s32n, "s2": s2,
              "s8k": s8k, "l2o": l2o, "perm": perm, "lf16": lf16,
              "rfill": rfill}
        if DR_PD:
            im["xt8"] = np.ascontiguousarray(
                xt.reshape(4, 2, 128, B).transpose(0, 2, 1, 3)
            ).astype(ml_dtypes.float8_e4m3)
            im["tc8"] = np.ascontiguousarray(
                tc_c.reshape(4, 2, 128, F).transpose(0, 2, 1, 3)
            ).astype(ml_dtypes.float8_e4m3)
            im["psel8"] = np.ascontiguousarray(
                psel.reshape(2, 64, NPAIR).transpose(1, 0, 2)
            ).astype(ml_dtypes.float8_e4m3)
        else:
            im["psel"] = psel.astype(ml_dtypes.float8_e4m3)
            im["xt8f"] = np.ascontiguousarray(
                xt.reshape(NCI, 128, B).transpose(1, 0, 2)
            ).astype(ml_dtypes.float8_e4m3)
            im["tc8f"] = tc_c.astype(ml_dtypes.float8_e4m3)
        in_maps.append(im)

    kw = {}
    if _trace:
        kw = dict(trace=True, tmpdir=_tmpdir)
    res = run_bass_kernel_spmd(nc, in_maps, list(range(NCORES)), **kw)

    jj = np.arange(B, dtype=np.float32)
    junk_col = (B - jj)[None, :]          # A-tile po junk = 128 - j
    i_map, o_map = c["i_map"], c["o_map"]
    # junk corrections apply only to A-tile rows/cols (D junk is exact 0)
    a_mask_o = np.zeros((OC, 1), np.float32)   # po rows: o = 8t + osub
    for t in range(NT):
        if TILE_KIND[t] == "A":
            a_mask_o[8 * t:8 * t + 8] = 1.0
    a_mask_rs = np.zeros((1, 64), np.float32)  # rs cols: c = 8ig + t
    for col in range(64):
        if TILE_KIND[col % 8] == "A":
            a_mask_rs[0, col] = 1.0
    o_b = np.empty((B, OUT), np.float32)
    for cr in range(NCORES):
        r = res.results[cr]
        po = r["po"] - junk_col * a_mask_o            # [64, 128] colsums
        ob_c = po.T.copy()                            # [j, o_local]
        rows = r["rs"] - (i_map + 1) * a_mask_rs      # rowsums minus junk
        np.add.at(ob_c, (i_map.ravel(), o_map.ravel()), rows.ravel())
        o_b[:, OC * cr:OC * (cr + 1)] = ob_c
    out = np.concatenate([x, o_b], axis=1)
    if _trace:
        return out, res
    return out


# revision 7
# speedup vs baseline: 1.0740x; 1.0056x over previous
"""MinibatchDiscrimination kernel for 8 Trainium2 NeuronCores.

reference:
    m = einsum('bi,iok->bok', x, T)          # B=128, IN=1024, OUT=512, K=16
    norm[i,j,o] = sum_k |m[j,o,k] - m[i,o,k]|
    o_b = sum_i exp(-norm) - 1               # [B, OUT]
    out = concat([x, o_b], axis=1)           # [128, 1536]

Sharding: each core owns OUT/8 = 64 output features (zero communication).
All device inputs are fp8e4 with T pre-scaled by 1/2 (the exp scale
undoes it); m-quantization error only shifts norms that are >= ~130,
where exp underflows to exactly 0, so the output is bit-exact.

The 8 f-tiles (128 f-rows = 8 o x 16 k each) are split between two
engine pipelines (TILE_KIND):

A-tiles ('A'): pair-diff on PE (m_bf.T @ psel, psel[b,(i,j)] = +1{b==i}
  - 1{b==j} over the 8128 strictly-upper pairs), |.| on ACT in
  [128, 1024] supers, k-reduce on PE per i with packed 32-row strip
  selectors (tile_position). Unwritten PSUM cells (j <= i) read exact 0
  -> exp 1.0, removed host-side via known junk counts.

D-tiles ('D'): no PE pair-diff. DVE computes mm[f, (i', j)] =
  max(m_i, m_j) with one broadcast tensor_tensor op per 16-i super
  (padded row layout), using norm = 2*sum_k max(m_i,m_j) - r_i - r_j.
  The k-reduce contracts the max plane; the -(r_i + r_j)/2 terms are
  added into PSUM by three wide rank matmuls per [128, 512] bank
  (rank-8 o-slot selector for r_j, a fixed permutation matmul over a
  DMA-gathered per-row r column for r_i, and a rank-16 +BIG fill that
  makes every j <= i junk cell underflow to exact 0 in
  exp(-4*P) = exp(-norm)). No host junk correction for D-tiles.

Shared: exp on ACT ([128, 512], scale -2 for A / -4 for D). Column
sums: one wide selector matmul per e-tile into PSUM [64, 512] (the 4
igroup blocks summed by a final DVE reduce). Row sums via DVE
tensor_reduce. Host combines with per-kind junk corrections.
"""

import numpy as np
import ml_dtypes

import concourse.bass as bass
import concourse.tile as tile
from concourse import mybir
from concourse.bass_utils import run_bass_kernel_spmd

BF16 = mybir.dt.bfloat16
F32 = mybir.dt.float32
FP8 = mybir.dt.float8e4
A = mybir.AluOpType
AF = mybir.ActivationFunctionType
DRM = mybir.MatmulPerfMode.DoubleRow

B = 128
IN = 1024
OUT = 512
K = 16
NCORES = 8
OC = OUT // NCORES       # 64
F = OC * K               # 1024
NT = F // 128            # 8 f-tiles
NCI = IN // 128          # 8 contraction chunks
NPAIR = (B * (B - 1)) // 2   # 8128 strictly-upper pairs
CHUNK = 512
NCHUNK = (NPAIR + CHUNK - 1) // CHUNK   # 16 (last = 448)
SUPER = 1024
NSUPER = (NPAIR + SUPER - 1) // SUPER   # 8 (last = 960)

# tile kinds: 'A' = PE pair-diff + ACT abs; 'D' = DVE minmax supers
TILE_KIND = "ADADADAA"
DR_PD = False           # (DoubleRow gave no HW rate gain)

# D-tile supers: igroup-aligned supers of 16 i's, padded row width
DSUP_W = [127 - 16 * s for s in range(8)]
DSUP_OFF = np.cumsum([0] + [16 * w for w in DSUP_W]).tolist()
DW = DSUP_OFF[-1]       # 16*(127+111+...+15) = 9088


def _pair_base(i):
    return i * 127 - (i * (i - 1)) // 2


def _split_excess_waits(nc, max_waits=1):
    """This walrus build rejects instructions carrying more than one sem
    wait; hoist extras onto preceding NoOps on the same engine."""
    for fn in nc.m.functions:
        for blk in fn.blocks:
            new_insts = []
            for inst in blk.instructions:
                si = inst.sync_info
                if si and si.on_wait and len(si.on_wait) > max_waits:
                    waits = list(si.on_wait)
                    extra, keep = waits[:-max_waits], waits[-max_waits:]
                    k = 0
                    while extra:
                        chunk, extra = extra[:max_waits], extra[max_waits:]
                        nop = mybir.InstNoOp(
                            name=f"{inst.name}-ws{k}", engine=inst.engine,
                            ins=[], outs=[],
                            sync_info=mybir.SyncInfo(on_wait=chunk, on_update=[]))
                        nc.register_instruction(nop)
                        new_insts.append(nop)
                        k += 1
                    inst.sync_info = mybir.SyncInfo(
                        on_wait=keep, on_update=list(si.on_update))
                new_insts.append(inst)
            blk.instructions[:] = new_insts


def _make_a_steps(nc, pools, t, m8, psel8, m_bf, psel_sb):
    """A-tile: pair-diff chunks (PE) + |.| (ACT) -> absd bf16."""
    work, ework, pdiff, pnorm = pools
    absd = work.tile([128, NPAIR], BF16, tag="absd")

    def step(c):
        lo = c * SUPER
        w = min(SUPER, NPAIR - lo)
        pd = pdiff.tile([128, SUPER], F32, tag="pd")
        for h in range(0, w, CHUNK):
            hw = min(CHUNK, w - h)
            if DR_PD:
                nc.tensor.matmul(
                    pd[:, h:h + hw], m8[:, :, 128 * t:128 * (t + 1)],
                    psel8[:, :, lo + h:lo + h + hw],
                    start=True, stop=True, perf_mode=DRM,
                    skip_group_check=True)
            else:
                nc.tensor.matmul(
                    pd[:, h:h + hw], m_bf[:, 128 * t:128 * (t + 1)],
                    psel_sb[:, lo + h:lo + h + hw],
                    start=True, stop=True)
        nc.scalar.activation(absd[:, lo:lo + w], pd[:, 0:w], AF.Abs)

    return absd, [lambda c=c: step(c) for c in range(NSUPER)]


def _make_d_steps(nc, pools, t, m_t):
    """D-tile (max form): mm[f, (i', j)] = max(m_i, m_j) on DVE, one
    broadcast op per 16-i super. norm = 2*sum_k max - r_i - r_j; the r
    terms and a +BIG junk fill are added into PSUM by rank matmuls."""
    work, ework, pdiff, pnorm = pools
    mm = work.tile([128, DW], BF16, tag="mm")

    def step(s):
        w = DSUP_W[s]
        off = DSUP_OFF[s]
        in0 = m_t[:, 16 * s:16 * s + 16].unsqueeze(2).broadcast_to(
            [128, 16, w])
        in1 = m_t[:, 16 * s + 1:128].unsqueeze(1).broadcast_to(
            [128, 16, w])
        nc.vector.tensor_tensor(
            mm[:, off:off + 16 * w].rearrange("p (a b) -> p a b", a=16),
            in0, in1, op=A.max)

    return mm, [lambda s=s: step(s) for s in range(8)]


def _emit_kred(nc, pools, t, kind, buf, s32_sb, s32n_sb, s2_sb, po, rs_all,
               weave=None, last_tile=False, raux=None):
    """k-reduce + exp + row/col sums for tile t (both kinds).
    `weave` interleaves the NEXT tile's production steps into the PE
    stream so its ACT/DVE work overlaps this tile's k-reduce."""
    work, ework, pdiff, pnorm = pools
    weave = list(weave or [])
    n_mm = 8 * 16 * (2 if kind == "D" else 1)
    stride = max(1, n_mm // (len(weave) + 1)) if weave else 0
    mm_count = 0

    def tick():
        nonlocal mm_count
        mm_count += 1
        if weave and stride and mm_count % stride == 0:
            weave.pop(0)()

    for G in range(2):
        pn = pnorm.tile([128, 512], F32, tag="pn")
        nc.vector.memset(pn[:], 0.0)
        for gl in range(4):
            ig = 4 * G + gl
            for idx in range(16):
                q, a = idx % 4, idx // 4
                i = 16 * ig + 4 * a + q
                if i >= B - 1:
                    continue
                w = 127 - i
                out_ap = pn[32 * q:32 * q + 32,
                            128 * gl + i + 1:128 * (gl + 1)]
                last = (gl == 3 and idx == 15) and kind == "A"
                if kind == "A":
                    bs = _pair_base(i)
                    nc.tensor.matmul(
                        out_ap, s32_sb[a][:], buf[:, bs:bs + w],
                        start=False, stop=last,
                        tile_position=(0, 32 * q), skip_group_check=True)
                    tick()
                else:
                    s = ig
                    isub = i - 16 * s
                    cs = DSUP_OFF[s] + isub * DSUP_W[s] + (i - 16 * s)
                    nc.tensor.matmul(
                        out_ap, s32_sb[a][:], buf[:, cs:cs + w],
                        start=False, stop=False,
                        tile_position=(0, 32 * q), skip_group_check=True)
                    tick()
        if kind == "D":
            l2o_sb, rtn, perm_sb, rpk2, lf16_sb, rfill_sb = raux
            # r_j: same [8,128] row tile for every gl block (0-stride rep)
            nc.tensor.matmul(
                pn[:], l2o_sb[:],
                rtn[:].unsqueeze(1).broadcast_to([8, 4, 128]),
                start=False, stop=False, skip_group_check=True)
            # r_i: per-igroup gathered column, broadcast along j
            nc.tensor.matmul(
                pn[:], perm_sb[:],
                rpk2[:, 4 * G:4 * G + 4].unsqueeze(2).broadcast_to(
                    [128, 4, 128]),
                start=False, stop=False, skip_group_check=True)
            # +BIG fill on j <= i cells
            nc.tensor.matmul(
                pn[:], lf16_sb[:],
                rfill_sb[:, 512 * G:512 * (G + 1)],
                start=False, stop=True, skip_group_check=True)
        e = ework.tile([128, 512], BF16, tag="e")
        # A-tiles: exp(-2*norm_half). D-tiles: PSUM P = sum_k max
        # - (r_i + r_j)/2 (+BIG on j<=i) and norm_half = 2P -> exp(-4P),
        # junk underflows to exactly 0 (no host correction).
        nc.scalar.activation(e[:], pn[:], AF.Exp,
                             scale=(-4.0 if kind == "D" else -2.0))
        rs_view = rs_all.rearrange("p (ig tt) -> p ig tt", tt=8)
        nc.vector.tensor_reduce(
            rs_view[:, 4 * G:4 * G + 4, t],
            e[:].rearrange("p (g j) -> p g j", g=4), op=A.add,
            axis=mybir.AxisListType.X)
        nc.tensor.matmul(po[:], s2_sb[t][:], e[:],
                         start=(t == 0 and G == 0),
                         stop=(last_tile and G == 1))
    for stp in weave:
        stp()


def _build_program():
    nc = bass.Bass()
    xT_d = nc.dram_tensor("xt", [IN, B], BF16, kind="ExternalInput")
    tc_d = nc.dram_tensor("tc", [IN, F], BF16, kind="ExternalInput")
    s32_d = nc.dram_tensor("s32", [4, 128, 32], BF16, kind="ExternalInput")
    s8k_d = nc.dram_tensor("s8k", [128, 8], BF16, kind="ExternalInput")
    l2o_d = nc.dram_tensor("l2o", [8, 128], BF16, kind="ExternalInput")
    perm_d = nc.dram_tensor("perm", [128, 128], BF16, kind="ExternalInput")
    lf16_d = nc.dram_tensor("lf16", [16, 128], BF16, kind="ExternalInput")
    rfill_d = nc.dram_tensor("rfill", [16, 8 * 128], BF16,
                             kind="ExternalInput")
    s32n_d = nc.dram_tensor("s32n", [4, 128, 32], BF16, kind="ExternalInput")
    s2_d = nc.dram_tensor("s2", [NT, 128, OC], BF16, kind="ExternalInput")
    po_d = nc.dram_tensor("po", [OC, B], F32, kind="ExternalOutput")
    rs_d = nc.dram_tensor("rs", [128, 64], F32, kind="ExternalOutput")
    xt8f_d = nc.dram_tensor("xt8f", [128, NCI, B], FP8, kind="ExternalInput")
    tc8f_d = nc.dram_tensor("tc8f", [IN, F], FP8, kind="ExternalInput")
    if DR_PD:
        # x arranged for DR GEMM: xt8[cp, p, kt, b] = x.T[256cp+128kt+p, b]
        xt8_d = nc.dram_tensor("xt8", [4, 128, 2, B], FP8,
                               kind="ExternalInput")
        tc8_d = nc.dram_tensor("tc8", [4, 128, 2, F], FP8,
                               kind="ExternalInput")
        psel8_d = nc.dram_tensor("psel8", [64, 2, NPAIR], FP8,
                                 kind="ExternalInput")
    else:
        psel_d = nc.dram_tensor("psel", [B, NPAIR], FP8,
                                kind="ExternalInput")

    a_tiles = [t for t in range(NT) if TILE_KIND[t] == "A"]
    d_tiles = [t for t in range(NT) if TILE_KIND[t] == "D"]

    with tile.TileContext(nc) as tc:
        with (
            tc.tile_pool(name="cst", bufs=1) as cst,
            tc.tile_pool(name="work", bufs=3) as work,
            tc.tile_pool(name="ework", bufs=6) as ework,
            tc.tile_pool(name="pdiff", bufs=2, space="PSUM") as pdiff,
            tc.tile_pool(name="pnorm", bufs=3, space="PSUM") as pnorm,
            tc.tile_pool(name="pob", bufs=1, space="PSUM") as pob,
        ):
            pools = (work, ework, pdiff, pnorm)

            # ---- constant loads ----
            s32_sb, s32n_sb = [], []
            for a in range(4):
                t_ = cst.tile([128, 32], BF16, tag=f"s32_{a}")
                nc.sync.dma_start(t_[:], s32_d[a])
                s32_sb.append(t_)
                t_ = cst.tile([128, 32], BF16, tag=f"s32n_{a}")
                nc.sync.dma_start(t_[:], s32n_d[a])
                s32n_sb.append(t_)
            s2_sb = []
            for t in range(NT):
                t_ = cst.tile([128, OC], BF16, tag=f"s2{t}")
                nc.sync.dma_start(t_[:], s2_d[t])
                s2_sb.append(t_)

            m_bf, psel_sb, m8, psel8 = None, None, None, None
            if DR_PD:
                xt8_sb, tc8_sb = [], []
                for cp in range(4):
                    t_ = cst.tile([128, 2, B], FP8, tag=f"xt8{cp}")
                    nc.sync.dma_start(t_[:], xt8_d[cp])
                    xt8_sb.append(t_)
                    t_ = cst.tile([128, 2, F], FP8, tag=f"tc8{cp}")
                    nc.sync.dma_start(t_[:], tc8_d[cp])
                    tc8_sb.append(t_)
                psel8 = cst.tile([64, 2, NPAIR], FP8, tag="psel8")
                for cch in range(NCHUNK):
                    lo = cch * CHUNK
                    w = min(CHUNK, NPAIR - lo)
                    nc.sync.dma_start(psel8[:, :, lo:lo + w],
                                      psel8_d[:, :, lo:lo + w])
                # ---- DR GEMM -> m8 [64, 2, F] fp8 ----
                m8 = cst.tile([64, 2, F], FP8, tag="m8")
                for half in range(2):
                    for H in range(2):
                        psf = pdiff.tile([128, SUPER], F32, tag="pd")
                        ps = psf[0:64, 0:512]
                        for cp in range(4):
                            nc.tensor.matmul(
                                ps,
                                xt8_sb[cp][:, :, 64 * H:64 * (H + 1)],
                                tc8_sb[cp][:, :, 512 * half:512 * (half + 1)],
                                start=(cp == 0), stop=(cp == 3),
                                perf_mode=DRM, skip_group_check=True)
                        nc.vector.tensor_copy(
                            m8[0:64, H, 512 * half:512 * (half + 1)], ps)
            else:
                xt_all = cst.tile([128, NCI, B], FP8, tag="xtall")
                nc.sync.dma_start(xt_all[:], xt8f_d[:])
                xT_sb = [xt_all[:, ci, :] for ci in range(NCI)]
                tcc_sb = []
                for ci in range(NCI):
                    t_ = cst.tile([128, F], FP8, tag=f"tcc{ci}")
                    nc.sync.dma_start(t_[:], tc8f_d[128 * ci:128 * (ci + 1), :])
                    tcc_sb.append(t_)
                psel_sb = cst.tile([128, NPAIR], FP8, tag="psel")
                nc.sync.dma_start(psel_sb[:, 0:4096], psel_d[:, 0:4096])
                nc.sync.dma_start(psel_sb[:, 4096:NPAIR],
                                  psel_d[:, 4096:NPAIR])
                m_bf = cst.tile([128, F], FP8, tag="mbf")
                for half in range(2):
                    psf = pdiff.tile([128, SUPER], F32, tag="pd")
                    ps = psf[:, 0:512]
                    for ci in range(NCI):
                        nc.tensor.matmul(
                            ps, xT_sb[ci],
                            tcc_sb[ci][:, 512 * half:512 * (half + 1)],
                            start=(ci == 0), stop=(ci == NCI - 1))
                    nc.vector.tensor_copy(
                        m_bf[:, 512 * half:512 * (half + 1)], ps)

            # ---- transposed GEMM for D-tiles: m_t [f, b] bf16 ----
            if d_tiles:
                s8k_sb = cst.tile([128, 8], BF16, tag="s8k")
                nc.sync.dma_start(s8k_sb[:], s8k_d[:])
                l2o_sb = cst.tile([8, 128], BF16, tag="l2o")
                nc.sync.dma_start(l2o_sb[:], l2o_d[:])
                perm_sb = cst.tile([128, 128], BF16, tag="perm")
                nc.sync.dma_start(perm_sb[:], perm_d[:])
                lf16_sb = cst.tile([16, 128], BF16, tag="lf16")
                nc.sync.dma_start(lf16_sb[:], lf16_d[:])
                rfill_sb = cst.tile([16, 8 * 128], BF16, tag="rfill")
                nc.sync.dma_start(rfill_sb[:], rfill_d[:])
                m_T, r_aux = {}, {}
                for t in d_tiles:
                    pmf = pdiff.tile([128, SUPER], F32, tag="pd")
                    pm = pmf[:, 0:128]
                    for ci in range(NCI):
                        nc.tensor.matmul(
                            pm, tcc_sb[ci][:, 128 * t:128 * (t + 1)],
                            xT_sb[ci],
                            start=(ci == 0), stop=(ci == NCI - 1))
                    mt = cst.tile([128, 128], BF16, tag=f"mt{t}")
                    nc.scalar.activation(mt[:], pm, AF.Copy, scale=1.0)
                    m_T[t] = mt
                    # rtn[osub, i] = -0.5 * sum_k m_t[(osub,k), i]
                    prf = pdiff.tile([128, SUPER], F32, tag="pd")
                    pr = prf[0:8, 0:128]
                    nc.tensor.matmul(pr, s8k_sb[:], mt[:],
                                     start=True, stop=True,
                                     skip_group_check=True)
                    rtn = cst.tile([8, 128], BF16, tag=f"rtn{t}")
                    nc.vector.tensor_scalar(rtn[:], pr, -0.5, None,
                                            op0=A.mult)
                    # rpk2[c= (osub,a,q)-major, ig] = rtn[osub, 16ig+4a+q]
                    rpk2 = cst.tile([128, 8], BF16, tag=f"rpk{t}")
                    for ig in range(8):
                        nc.sync.dma_start(
                            rpk2[:, ig:ig + 1],
                            rtn[:, 16 * ig:16 * ig + 16].unsqueeze(2))
                    r_aux[t] = (rtn, rpk2)

            po = pob.tile([OC, 512], F32, tag="po")
            rs_all = cst.tile([128, 64], F32, tag="rs")

            # ---- software pipeline over tiles ----
            def make_steps(t):
                if TILE_KIND[t] == "A":
                    return _make_a_steps(nc, pools, t, m8, psel8,
                                         m_bf, psel_sb)
                return _make_d_steps(nc, pools, t, m_T[t])

            cur_buf, steps0 = make_steps(0)
            for s in steps0:
                s()
            for t in range(NT):
                if t + 1 < NT:
                    nxt_buf, nxt_steps = make_steps(t + 1)
                else:
                    nxt_buf, nxt_steps = None, []
                if TILE_KIND[t] == "D":
                    rtn, rpk2 = r_aux[t]
                    raux = (l2o_sb, rtn, perm_sb, rpk2, lf16_sb, rfill_sb)
                else:
                    raux = None
                _emit_kred(nc, pools, t, TILE_KIND[t], cur_buf,
                           s32_sb, s32n_sb, s2_sb, po, rs_all,
                           weave=nxt_steps, last_tile=(t == NT - 1),
                           raux=raux)
                cur_buf = nxt_buf

            po_sb = cst.tile([OC, B], F32, tag="posb")
            # sum the 4 igroup column blocks: view [o, j, g] so axis=X
            # reduces g (stride-128 middle dim moved last)
            nc.vector.tensor_reduce(
                po_sb[:],
                po[:].rearrange("o (g j) -> o j g", g=4),
                op=A.add, axis=mybir.AxisListType.X)
            nc.sync.dma_start(po_d[:], po_sb[:])
            nc.sync.dma_start(rs_d[:], rs_all[:])

    _split_excess_waits(nc)
    return nc


def _host_consts():
    psel = np.zeros((B, NPAIR), np.float32)
    col = 0
    for i in range(B - 1):
        w = 127 - i
        psel[i, col:col + w] = 1.0
        psel[np.arange(i + 1, 128), np.arange(col, col + w)] = -1.0
        col += w
    s32 = np.zeros((4, 128, 32), np.float32)
    for a in range(4):
        for osub in range(8):
            s32[a, 16 * osub:16 * (osub + 1), 8 * a + osub] = 1.0
    s2 = np.zeros((NT, 128, OC), np.float32)
    for t in range(NT):
        for p in range(128):
            s2[t, p, 8 * t + (p % 8)] = 1.0
    s8k = np.zeros((128, 8), np.float32)
    for f in range(128):
        s8k[f, f // 16] = 1.0
    l2o = np.zeros((8, 128), np.float32)
    for p in range(128):
        l2o[p % 8, p] = 1.0
    perm = np.zeros((128, 128), np.float32)
    for cc in range(128):
        osub, rem = cc // 16, cc % 16
        a_, q_ = rem // 4, rem % 4
        perm[cc, 32 * q_ + 8 * a_ + osub] = 1.0
    BIG = 30000.0
    lf16 = np.zeros((16, 128), np.float32)
    for p in range(128):
        q_, rem = p // 32, p % 32
        a_ = rem // 8
        lf16[4 * a_ + q_, p] = 1.0
    rfill = np.zeros((16, 8 * 128), np.float32)
    for ig in range(8):
        for sp in range(16):
            a_, q_ = sp // 4, sp % 4
            i = 16 * ig + 4 * a_ + q_
            rfill[sp, 128 * ig:128 * ig + i + 1] = BIG
    bf = ml_dtypes.bfloat16
    return (psel, s32.astype(bf), s2.astype(bf),
            s8k.astype(bf), l2o.astype(bf), perm.astype(bf),
            lf16.astype(bf), rfill.astype(bf))


_CACHE = {}


def _get_cached():
    if "nc" not in _CACHE:
        _CACHE["nc"] = _build_program()
        _CACHE["consts"] = _host_consts()
        p_idx = np.arange(128)
        q, rem = p_idx // 32, p_idx % 32
        a_, osub = rem // 8, rem % 8
        cols = np.arange(64)
        ig, t_ = cols // 8, cols % 8
        i_map = 16 * ig[None, :] + 4 * a_[:, None] + q[:, None]   # [128, 64]
        o_map = 8 * t_[None, :] + osub[:, None]                   # [128, 64]
        _CACHE["i_map"] = i_map
        _CACHE["o_map"] = o_map
    return _CACHE


def kernel(x: np.ndarray, T: np.ndarray, _trace=False, _tmpdir=None) -> np.ndarray:
    x = np.asarray(x, dtype=np.float32)
    T = np.asarray(T, dtype=np.float32)
    c = _get_cached()
    nc = c["nc"]
    (psel, s32, s2, s8k, l2o, perm, lf16, rfill) = c["consts"]
    psel = np.asarray(psel, np.float32)
    s32n = (-np.asarray(s32, np.float32)).astype(ml_dtypes.bfloat16)

    xt = np.ascontiguousarray(x.T)
    in_maps = []
    for cr in range(NCORES):
        # T scaled by 1/2 so m fits fp8e4 and diffs stay in range
        tc_c = np.ascontiguousarray(
            (0.5 * T[:, OC * cr:OC * (cr + 1), :]).reshape(IN, F))
        im = {"xt": xt.astype(ml_dtypes.bfloat16),
              "tc": tc_c.astype(ml_dtypes.bfloat16),
              "s32": s32, "s32n": s32n, "s2": s2,
              "s8k": s8k, "l2o": l2o, "perm": perm, "lf16": lf16,
              "rfill": rfill}
        if DR_PD:
            im["xt8"] = np.ascontiguousarray(
                xt.reshape(4, 2, 128, B).transpose(0, 2, 1, 3)
            ).astype(ml_dtypes.float8_e4m3)
            im["tc8"] = np.ascontiguousarray(
                tc_c.reshape(4, 2, 128, F).transpose(0, 2, 1, 3)
            ).astype(ml_dtypes.float8_e4m3)
            im["psel8"] = np.ascontiguousarray(
                psel.reshape(2, 64, NPAIR).transpose(1, 0, 2)
            ).astype(ml_dtypes.float8_e4m3)
        else:
            im["psel"] = psel.astype(ml_dtypes.float8_e4m3)
            im["xt8f"] = np.ascontiguousarray(
                xt.reshape(NCI, 128, B).transpose(1, 0, 2)
            ).astype(ml_dtypes.float8_e4m3)
            im["tc8f"] = tc_c.astype(ml_dtypes.float8_e4m3)
        in_maps.append(im)

    kw = {}
    if _trace:
        kw = dict(trace=True, tmpdir=_tmpdir)
    res = run_bass_kernel_spmd(nc, in_maps, list(range(NCORES)), **kw)

    jj = np.arange(B, dtype=np.float32)
    junk_col = (B - jj)[None, :]          # A-tile po junk = 128 - j
    i_map, o_map = c["i_map"], c["o_map"]
    # junk corrections apply only to A-tile rows/cols (D junk is exact 0)
    a_mask_o = np.zeros((OC, 1), np.float32)   # po rows: o = 8t + osub
    for t in range(NT):
        if TILE_KIND[t] == "A":
            a_mask_o[8 * t:8 * t + 8] = 1.0
    a_mask_rs = np.zeros((1, 64), np.float32)  # rs cols: c = 8ig + t
    for col in range(64):
        if TILE_KIND[col % 8] == "A":
            a_mask_rs[0, col] = 1.0
    o_b = np.empty((B, OUT), np.float32)
    for cr in range(NCORES):
        r = res.results[cr]
        po = r["po"] - junk_col * a_mask_o            # [64, 128] colsums
        ob_c = po.T.copy()                            # [j, o_local]
        rows = r["rs"] - (i_map + 1) * a_mask_rs      # rowsums minus junk
        np.add.at(ob_c, (i_map.ravel(), o_map.ravel()), rows.ravel())
        o_b[:, OC * cr:OC * (cr + 1)] = ob_c
    out = np.concatenate([x, o_b], axis=1)
    if _trace:
        return out, res
    return out
